# revision 1
# baseline (speedup 1.0000x reference)
"""Distributed Trainium2 Bass kernel for AdaptiveGCN (N=4096, CIN=1024, H=1024, COUT=512, R=10).

Sharding: node dimension split across 8 NeuronCores (512 nodes/core).
Each core owns a 512-column block of the dense adaptive adjacency and the
matching 512 output rows. Collectives (all AllGathers, explicitly ordered on
the single CC stream so the tiny dinv gather slots between the XW1 quarter
gathers): softmax row-sum partials (16KB), dinv (2KB), XW1 in four bf16
quarters (256KB each), XW2 in four bf16 quarters (128KB each). The final
mean-pool partial is returned per core and reduced on the host along with
the scalar attention gate.

kernel(**inputs) takes the FULL unsharded inputs (same keys as
reference.setup_inputs()) and returns the FULL [1, 512] float32 output.
"""

import os
import sys
from contextlib import ExitStack

import numpy as np

for _p in ("/opt/trn_rl_repo", "/root/.axon_site/_ro/trn_rl_repo"):
    if os.path.isdir(_p) and _p not in sys.path:
        sys.path.insert(0, _p)

import concourse.bass as bass
import concourse.bacc as bacc
import concourse.tile as tile
from concourse import mybir
from concourse.bass_utils import run_bass_kernel_spmd
from concourse.masks import make_identity
from concourse.tile_rust import add_dep_helper

F32 = mybir.dt.float32
F32R = mybir.dt.float32r
BF16 = mybir.dt.bfloat16
AF = mybir.ActivationFunctionType
OP = mybir.AluOpType
AX = mybir.AxisListType

NCORES = 8
N = 4096
NL = N // NCORES          # 512 nodes per core
CIN = 1024
H = 1024
CO = 512
R = 10
P = 128
JT = N // P               # 32 j-tiles
ET = H // P               # 8
IT = NL // P              # 4 local-node tiles
GT = CO // P              # 4
KC = CIN // P             # 8 cin k-tiles
BN_EPS = 1e-5
NQ = 2                    # AG half count
HQ = H // NQ              # 512: AG1 half width (4 f-tiles)
CQ = CO // NQ             # 256: AG2 half width (2 g-tiles)


def build():
    """Build the SPMD Bass graph (identical on all 8 cores)."""
    nc = bacc.Bacc(None, target_bir_lowering=False, debug=False, num_devices=NCORES)

    # ---- external parameters (per-core shards / replicated) ----
    xT_d = nc.declare_dram_parameter("xT", [CIN, NL], BF16, isOutput=False)
    wmap_d = nc.declare_dram_parameter("wmap", [CIN, H], BF16, isOutput=False)
    w1_d = nc.declare_dram_parameter("w1", [H, H], BF16, isOutput=False)
    w2_d = nc.declare_dram_parameter("w2", [H, CO], BF16, isOutput=False)
    nv1T_d = nc.declare_dram_parameter("nv1T", [R, N], F32R, isOutput=False)
    nv2s_d = nc.declare_dram_parameter("nv2s", [R, NL], F32R, isOutput=False)
    bmap_d = nc.declare_dram_parameter("bmap_t", [P, ET], F32, isOutput=False)
    b1_d = nc.declare_dram_parameter("b1_t", [P, ET], F32, isOutput=False)
    s1_d = nc.declare_dram_parameter("s1_t", [P, ET], F32, isOutput=False)
    t1_d = nc.declare_dram_parameter("t1_t", [P, ET], F32, isOutput=False)
    b2_d = nc.declare_dram_parameter("b2_t", [P, GT], F32, isOutput=False)
    s2_d = nc.declare_dram_parameter("s2_t", [P, GT], F32, isOutput=False)
    t2_d = nc.declare_dram_parameter("t2_t", [P, GT], F32, isOutput=False)
    out_d = nc.declare_dram_parameter("out", [P, GT], F32, isOutput=True)

    # ---- internal DRAM bounce buffers for collectives ----
    rg = [list(range(NCORES))]
    rs_in = nc.dram_tensor("rs_in", [P, JT], F32)
    rs_out = nc.dram_tensor("rs_out", [NCORES * P, JT], F32, addr_space="Shared")
    dv_in = nc.dram_tensor("dv_in", [NL], F32)
    dv_out = nc.dram_tensor("dv_out", [N], F32, addr_space="Shared")
    xw1_in = [nc.dram_tensor(f"xw1_in{q}", [NL, HQ], BF16) for q in range(NQ)]
    xw1_out = [
        nc.dram_tensor(f"xw1_out{q}", [N, HQ], BF16, addr_space="Shared")
        for q in range(NQ)
    ]
    xw2_in = [nc.dram_tensor(f"xw2_in{q}", [NL, CQ], BF16) for q in range(NQ)]
    xw2_out = [
        nc.dram_tensor(f"xw2_out{q}", [N, CQ], BF16, addr_space="Shared")
        for q in range(NQ)
    ]

    cc_insts = []

    def collective(in_ap, out_ap):
        cc = nc.gpsimd.collective_compute(
            "AllGather", OP.bypass, replica_groups=rg,
            ins=[in_ap], outs=[out_ap],
        )
        cc_insts.append(cc)
        return cc

    with tile.TileContext(nc) as tc:
        with ExitStack() as ctx:
            # ---------- persistent pool ----------
            pp = ctx.enter_context(tc.tile_pool(name="persist", bufs=1))

            nv1T_sb = pp.tile([R, N], F32R)
            nv2s_sb = pp.tile([R, NL], F32R)
            nc.sync.dma_start(nv1T_sb[:], nv1T_d[:])
            nc.sync.dma_start(nv2s_sb[:], nv2s_d[:])

            rs_part = pp.tile([P, JT], F32)
            rs_all = pp.tile([P, NCORES * JT], F32)
            rowsum_sb = pp.tile([P, JT], F32)
            r_sb = pp.tile([P, JT], F32)
            r_bf = pp.tile([P, JT], BF16)
            rdv_sb = pp.tile([P, JT], F32)
            dinvt_sb = pp.tile([P, JT], F32)
            dinv_rep = pp.tile([P, NL], F32)
            dinv_loc = pp.tile([1, NL], F32)
            degs = pp.tile([1, NL], F32)

            bmap_sb = pp.tile([P, ET], F32)
            b1_sb = pp.tile([P, ET], F32)
            s1_sb = pp.tile([P, ET], F32)
            t1_sb = pp.tile([P, ET], F32)
            b2_sb = pp.tile([P, GT], F32)
            s2_sb = pp.tile([P, GT], F32)
            t2_sb = pp.tile([P, GT], F32)
            for sb, d in (
                (bmap_sb, bmap_d), (b1_sb, b1_d), (s1_sb, s1_d), (t1_sb, t1_d),
                (b2_sb, b2_d), (s2_sb, s2_d), (t2_sb, t2_d),
            ):
                nc.sync.dma_start(sb[:], d[:])

            # expz starts as exp(relu(z)) and is scaled IN PLACE into
            # M[j, i] = dinv[j] * adp[j, i] after the collectives.
            M_sb = pp.tile([P, JT * NL], BF16)
            diag_sb = pp.tile([P, IT * NL], BF16)     # self-loop rhs tiles
            ident_sb = pp.tile([P, P], F32)
            make_identity(nc, ident_sb[:])

            h1T_sb = pp.tile([P, ET * NL], BF16)      # conv1 out, [f, i] layout
            h2T_sb = pp.tile([P, GT * NL], BF16)      # conv2 out, [g, i] layout
            xw1bf_sb = pp.tile([P, IT * H], BF16)     # local XW1, [i, f] layout
            xw2bf_sb = pp.tile([P, IT * CO], BF16)    # local XW2, [i, g] layout
            pool_part = pp.tile([P, GT], F32)
            w2_sb = pp.tile([P, KC * CO], BF16)
            for kt in range(KC):
                nc.sync.dma_start(
                    w2_sb[:, kt * CO:(kt + 1) * CO],
                    w2_d[kt * P:(kt + 1) * P, :],
                )

            # ---------- phase A: adjacency column block + feature chain ----
            with ExitStack() as actx:
                pa = actx.enter_context(tc.tile_pool(name="phaseA", bufs=1))
                tmp_pool = actx.enter_context(tc.tile_pool(name="tmpA", bufs=4))
                psA = actx.enter_context(
                    tc.tile_pool(name="psA", bufs=3, space="PSUM")
                )
                psC = actx.enter_context(
                    tc.tile_pool(name="psC", bufs=1, space="PSUM")
                )

                # z = nv1 @ nv2 column block; relu; exp with fused row-sum
                for jt in range(JT):
                    zp = psA.tile([P, NL], F32, tag="zp", name=f"zp{jt}")
                    nc.tensor.matmul(
                        zp[:],
                        nv1T_sb[:, jt * P:(jt + 1) * P],
                        nv2s_sb[:],
                        start=True, stop=True,
                    )
                    zr = tmp_pool.tile([P, NL], F32, tag="zr", name=f"zr{jt}")
                    nc.vector.tensor_scalar_max(zr[:], zp[:], 0.0)
                    nc.scalar.activation(
                        M_sb[:, jt * NL:(jt + 1) * NL], zr[:], AF.Exp,
                        accum_out=rs_part[:, jt:jt + 1],
                    )

                # AllGather the per-core softmax row-sum partials, sum locally
                nc.gpsimd.dma_start(rs_in[:], rs_part[:])
                collective(rs_in[:], rs_out[:])
                nc.gpsimd.dma_start(
                    rs_all[:].rearrange("p (c t) -> p c t", t=JT),
                    rs_out.rearrange("(c p) t -> p c t", p=P),
                )
                nc.vector.tensor_add(
                    rowsum_sb[:], rs_all[:, 0:JT], rs_all[:, JT:2 * JT]
                )
                for c in range(2, NCORES):
                    nc.vector.tensor_add(
                        rowsum_sb[:], rowsum_sb[:],
                        rs_all[:, c * JT:(c + 1) * JT],
                    )
                nc.vector.reciprocal(r_sb[:], rowsum_sb[:])
                nc.vector.tensor_copy(r_bf[:], r_sb[:])

                # ---------- feature mapping xmT = relu(wmap^T x^T + b) ------
                with ExitStack() as bctx:
                    pb = bctx.enter_context(tc.tile_pool(name="phaseB", bufs=1))
                    psB = bctx.enter_context(
                        tc.tile_pool(name="psB", bufs=2, space="PSUM")
                    )
                    xT_sb = pb.tile([P, KC * NL], BF16)
                    wm_sb = pb.tile([P, KC * H], BF16)
                    w1_sb = pb.tile([P, KC * H], BF16)
                    for kt in range(KC):
                        nc.sync.dma_start(
                            xT_sb[:, kt * NL:(kt + 1) * NL],
                            xT_d[kt * P:(kt + 1) * P, :],
                        )
                    for kt in range(KC):
                        nc.sync.dma_start(
                            wm_sb[:, kt * H:(kt + 1) * H],
                            wmap_d[kt * P:(kt + 1) * P, :],
                        )
                    for kt in range(KC):
                        nc.sync.dma_start(
                            w1_sb[:, kt * H:(kt + 1) * H],
                            w1_d[kt * P:(kt + 1) * P, :],
                        )
                    xmT_sb = pa.tile([P, ET * NL], BF16)
                    for et in range(ET):
                        mp = psB.tile([P, NL], F32, tag="mp", name=f"mp{et}")
                        for kt in range(KC):
                            nc.tensor.matmul(
                                mp[:],
                                wm_sb[:, kt * H + et * P: kt * H + (et + 1) * P],
                                xT_sb[:, kt * NL:(kt + 1) * NL],
                                start=(kt == 0), stop=(kt == KC - 1),
                            )
                        nc.scalar.activation(
                            xmT_sb[:, et * NL:(et + 1) * NL], mp[:], AF.Relu,
                            bias=bmap_sb[:, et:et + 1],
                        )

                    # ---- XW1 = xm @ w1 in four f-quarters, AllGather each --
                    for q in range(NQ):
                        for it in range(IT):
                            wp = psB.tile([P, HQ], F32, tag="mp",
                                          name=f"wp{q}{it}")
                            for kt in range(KC):
                                nc.tensor.matmul(
                                    wp[:],
                                    xmT_sb[:, kt * NL + it * P: kt * NL + (it + 1) * P],
                                    w1_sb[:, kt * H + q * HQ: kt * H + (q + 1) * HQ],
                                    start=(kt == 0), stop=(kt == KC - 1),
                                )
                            nc.vector.tensor_copy(
                                xw1bf_sb[:, it * H + q * HQ: it * H + (q + 1) * HQ],
                                wp[:],
                            )
                            nc.scalar.dma_start(
                                xw1_in[q][it * P:(it + 1) * P, :],
                                xw1bf_sb[:, it * H + q * HQ: it * H + (q + 1) * HQ],
                            )
                        if q == 0:
                            collective(xw1_in[0][:], xw1_out[0][:])

                    # ---------- degree, dinv ----------
                    csp = psC.tile([1, NL], F32)
                    for jt in range(JT):
                        nc.tensor.matmul(
                            csp[:],
                            r_bf[:, jt:jt + 1],
                            M_sb[:, jt * NL:(jt + 1) * NL],
                            start=(jt == 0), stop=(jt == JT - 1),
                        )
                    # dinv = 1/sqrt(colsum + 1)
                    nc.scalar.activation(degs[:], csp[:], AF.Sqrt, bias=1.0)
                    nc.vector.reciprocal(dinv_loc[:], degs[:])
                    nc.gpsimd.dma_start(dv_in[:], dinv_loc[:])
                    collective(dv_in[:], dv_out[:])
                    # remaining XW1 quarters after the tiny dinv gather
                    for q in range(1, NQ):
                        collective(xw1_in[q][:], xw1_out[q][:])

                nc.gpsimd.dma_start(
                    dinvt_sb[:], dv_out.rearrange("(t p) -> p t", p=P)
                )
                nc.gpsimd.dma_start(
                    dinv_rep[:], dv_in[None, :].to_broadcast((P, NL))
                )
                nc.vector.tensor_mul(rdv_sb[:], r_sb[:], dinvt_sb[:])

                # M[j, i] = dinv[j] * adp[j, i]  (in-place scale of exp block)
                for jt in range(JT):
                    nc.vector.tensor_scalar_mul(
                        M_sb[:, jt * NL:(jt + 1) * NL],
                        M_sb[:, jt * NL:(jt + 1) * NL],
                        rdv_sb[:, jt:jt + 1],
                    )
                # diag tiles: dinv[i] on the local diagonal (self-loop rhs)
                nc.gpsimd.memset(diag_sb[:], 0.0)
                for it in range(IT):
                    nc.vector.tensor_mul(
                        diag_sb[:, it * NL + it * P: it * NL + (it + 1) * P],
                        ident_sb[:],
                        dinv_rep[:, it * P:(it + 1) * P],
                    )

            # ---------- conv passes ----------
            def conv_pass(mts, slab_pool, ps_pool, xw_out_q, width,
                          off_f, xwbf, xwbf_stride, tagp):
                """An^T @ XW for output feature tiles `mts` using the gathered
                quarter `xw_out_q` ([N, width] bf16). Returns psum tiles."""
                psums = {
                    mt: ps_pool.tile([P, NL], F32, tag=f"{tagp}{mt}",
                                     name=f"{tagp}{mt}")
                    for mt in mts
                }
                for kt in range(JT):
                    slab = slab_pool.tile([P, width], BF16, tag=f"sl{tagp}",
                                          name=f"sl{tagp}{kt}")
                    nc.sync.dma_start(
                        slab[:], xw_out_q[kt * P:(kt + 1) * P, :]
                    )
                    for mt in mts:
                        fo = mt * P - off_f
                        nc.tensor.matmul(
                            psums[mt][:],
                            slab[:, fo:fo + P],
                            M_sb[:, kt * NL:(kt + 1) * NL],
                            start=(kt == 0), stop=False,
                        )
                for it in range(IT):
                    for mt in mts:
                        nc.tensor.matmul(
                            psums[mt][:],
                            xwbf[:, it * xwbf_stride + mt * P:
                                 it * xwbf_stride + (mt + 1) * P],
                            diag_sb[:, it * NL:(it + 1) * NL],
                            start=False, stop=(it == IT - 1),
                        )
                return psums

            def conv_epilogue(mts, psums, etmp_pool, b_sb, s_sb, t_sb, hT,
                              tagp, do_pool=False):
                for mt in mts:
                    ta = etmp_pool.tile([P, NL], F32, tag=f"ea{tagp}",
                                        name=f"ea{tagp}{mt}")
                    nc.vector.tensor_mul(ta[:], psums[mt][:], dinv_rep[:])
                    tb = etmp_pool.tile([P, NL], F32, tag=f"eb{tagp}",
                                        name=f"eb{tagp}{mt}")
                    nc.scalar.activation(
                        tb[:], ta[:], AF.Relu, bias=b_sb[:, mt:mt + 1]
                    )
                    nc.vector.tensor_scalar(
                        hT[:, mt * NL:(mt + 1) * NL], tb[:],
                        s_sb[:, mt:mt + 1], t_sb[:, mt:mt + 1],
                        op0=OP.mult, op1=OP.add,
                    )
                    if do_pool:
                        nc.vector.reduce_sum(
                            pool_part[:, mt:mt + 1],
                            hT[:, mt * NL:(mt + 1) * NL],
                            axis=AX.X,
                        )

            with ExitStack() as cctx:
                slab_pool = cctx.enter_context(tc.tile_pool(name="slab1", bufs=6))
                etmp = cctx.enter_context(tc.tile_pool(name="etmp", bufs=2))
                for q in range(NQ):
                    with ExitStack() as cq:
                        psq = cq.enter_context(
                            tc.tile_pool(name=f"ps1{q}", bufs=1, space="PSUM")
                        )
                        mts = range(4 * q, 4 * q + 4)
                        psums = conv_pass(mts, slab_pool, psq, xw1_out[q],
                                          HQ, q * HQ, xw1bf_sb, H, f"c1{q}")
                        conv_epilogue(mts, psums, etmp, b1_sb, s1_sb,
                                      t1_sb, h1T_sb, f"1{q}")

                # ---- XW2 = h1 @ w2, cast bf16, AllGather in four quarters --
                with ExitStack() as dctx:
                    ps2 = dctx.enter_context(
                        tc.tile_pool(name="ps2", bufs=2, space="PSUM")
                    )
                    for it in range(IT):
                        wp2 = ps2.tile([P, CO], F32, tag="wp2",
                                       name=f"wp2{it}")
                        for kt in range(ET):
                            nc.tensor.matmul(
                                wp2[:],
                                h1T_sb[:, kt * NL + it * P: kt * NL + (it + 1) * P],
                                w2_sb[:, kt * CO:(kt + 1) * CO],
                                start=(kt == 0), stop=(kt == ET - 1),
                            )
                        nc.vector.tensor_copy(
                            xw2bf_sb[:, it * CO:(it + 1) * CO], wp2[:]
                        )
                        for q in range(NQ):
                            nc.scalar.dma_start(
                                xw2_in[q][it * P:(it + 1) * P, :],
                                xw2bf_sb[:, it * CO + q * CQ:
                                         it * CO + (q + 1) * CQ],
                            )
                    for q in range(NQ):
                        collective(xw2_in[q][:], xw2_out[q][:])

            # ---------- conv2 (four passes over g quarters) + mean-pool ----
            with ExitStack() as ectx:
                slab2_pool = ectx.enter_context(tc.tile_pool(name="slab2", bufs=6))
                etmp2 = ectx.enter_context(tc.tile_pool(name="etmp2", bufs=2))
                for q in range(NQ):
                    with ExitStack() as cq:
                        psq = cq.enter_context(
                            tc.tile_pool(name=f"ps3{q}", bufs=1, space="PSUM")
                        )
                        mts = range(2 * q, 2 * q + 2)
                        psums = conv_pass(mts, slab2_pool, psq, xw2_out[q],
                                          CQ, q * CQ, xw2bf_sb, CO, f"c2{q}")
                        conv_epilogue(mts, psums, etmp2, b2_sb, s2_sb,
                                      t2_sb, h2T_sb, f"2{q}", do_pool=True)

            # per-core pooled partial out; host reduces across cores
            nc.gpsimd.dma_start(out_d[:], pool_part[:])

        # pin the CC stream order: rs, xw1_q0, dinv, xw1_q1..3, xw2_q0..3
        for a, b in zip(cc_insts[1:], cc_insts[:-1]):
            add_dep_helper(a.ins, b.ins, True, "cc stream order")

    nc.compile()
    return nc


_NC_CACHE = {}


def _get_nc():
    if "nc" not in _NC_CACHE:
        _NC_CACHE["nc"] = build()
    return _NC_CACHE["nc"]


def make_in_maps(inputs):
    import ml_dtypes

    f = np.float32
    bf = ml_dtypes.bfloat16
    x = np.asarray(inputs["x"], dtype=f)
    w_map = np.asarray(inputs["w_map"], dtype=f)
    w1 = np.asarray(inputs["w1"], dtype=f)
    w2 = np.asarray(inputs["w2"], dtype=f)
    nv1 = np.asarray(inputs["nv1"], dtype=f)
    nv2 = np.asarray(inputs["nv2"], dtype=f)

    def vec_t(v, nt):
        return np.ascontiguousarray(np.asarray(v, dtype=f).reshape(nt, P).T)

    s1 = (np.asarray(inputs["bn1_g"], f)
          / np.sqrt(np.asarray(inputs["bn1_v"], f) + BN_EPS))
    t1 = np.asarray(inputs["bn1_b"], f) - np.asarray(inputs["bn1_m"], f) * s1
    s2 = (np.asarray(inputs["bn2_g"], f)
          / np.sqrt(np.asarray(inputs["bn2_v"], f) + BN_EPS))
    t2 = np.asarray(inputs["bn2_b"], f) - np.asarray(inputs["bn2_m"], f) * s2

    common = {
        "wmap": np.ascontiguousarray(w_map.astype(bf)),
        "w1": np.ascontiguousarray(w1.astype(bf)),
        "w2": np.ascontiguousarray(w2.astype(bf)),
        "nv1T": np.ascontiguousarray(nv1.T),
        "bmap_t": vec_t(inputs["b_map"], ET),
        "b1_t": vec_t(inputs["b1"], ET),
        "s1_t": vec_t(s1, ET),
        "t1_t": vec_t(t1, ET),
        "b2_t": vec_t(inputs["b2"], GT),
        "s2_t": vec_t(s2, GT),
        "t2_t": vec_t(t2, GT),
    }
    in_maps = []
    for c in range(NCORES):
        m = dict(common)
        m["xT"] = np.ascontiguousarray(x[c * NL:(c + 1) * NL].T.astype(bf))
        m["nv2s"] = np.ascontiguousarray(nv2[:, c * NL:(c + 1) * NL])
        in_maps.append(m)
    return in_maps


def finish_host(results, inputs):
    """Sum per-core pooled partials, apply mean + attention gate."""
    f = np.float32
    pooled_sum = np.zeros(CO, f)
    for res in results:
        arr = np.asarray(res["out"], dtype=f)      # [P, GT], g = t*P + p
        pooled_sum += arr.T.reshape(-1)
    pooled = pooled_sum / N
    w_attn = np.asarray(inputs["w_attn"], f).reshape(-1)
    b_attn = np.asarray(inputs["b_attn"], f).reshape(-1)[0]
    z = float(pooled @ w_attn + b_attn)
    attn = 1.0 / (1.0 + np.exp(-z))
    return (pooled * attn)[None, :].astype(f)


def run(inputs, trace=False, tmpdir=None):
    nc = _get_nc()
    in_maps = make_in_maps(inputs)
    res = run_bass_kernel_spmd(
        nc, in_maps, core_ids=list(range(NCORES)), trace=trace, tmpdir=tmpdir
    )
    out = finish_host(res.results, inputs)
    return out, res


def kernel(**inputs):
    out, _ = run(inputs)
    return out



# revision 2
# speedup vs baseline: 1.1137x; 1.1137x over previous
"""Distributed Trainium2 Bass kernel for AdaptiveGCN (N=4096, CIN=1024, H=1024, COUT=512, R=10).

Sharding: node dimension split across 8 NeuronCores (512 nodes/core).
Each core owns a 512-column block of the dense adaptive adjacency and the
matching 512 output rows.

v2 layout of the math (per core):
  E        = exp(relu(nv1 @ nv2))            column block, bf16
  r_j      = 1 / rowsum(E)                   (rowsum partials AllGathered)
  M8       = fp8(128 * r_j * E)              adjacency block for fp8 DoubleRow
  deg_i    = colsum(r_j E) + 1  (local),  dinv_i = deg_i^-1/2   (local, NO gather)
  Y1       = fp8(4 * dinv_i * (xm @ w1))     scaled BEFORE the AllGather, fp8 wire
  conv1    = relu((M8^T @ Y1 + 128*Y1_self) * dinv_i / 512 + b1) -> bn1 -> h1
  Y2       = fp8(8 * dinv_i * (h1 @ w2))     single fp8 AllGather
  conv2    = same shape, then mean-pool partials returned per core.

Collectives (CC stream order): rowsum partials (16KB f32), Y1 in two fp8
halves (256KB each), Y2 in one fp8 shot (256KB). The dinv gather of the
baseline is gone (dinv is folded into Y before the gather).

kernel(**inputs) takes the FULL unsharded inputs (same keys as
reference.setup_inputs()) and returns the FULL [1, 512] float32 output.
"""

import os
import sys
from contextlib import ExitStack

import numpy as np

for _p in ("/opt/trn_rl_repo", "/root/.axon_site/_ro/trn_rl_repo"):
    if os.path.isdir(_p) and _p not in sys.path:
        sys.path.insert(0, _p)

import concourse.bass as bass
import concourse.bacc as bacc
import concourse.tile as tile
from concourse import mybir
from concourse.bass_utils import run_bass_kernel_spmd
from concourse.masks import make_identity
from concourse.tile_rust import add_dep_helper

F32 = mybir.dt.float32
BF16 = mybir.dt.bfloat16
F8 = mybir.dt.float8e4
AF = mybir.ActivationFunctionType
OP = mybir.AluOpType
AX = mybir.AxisListType
DR = mybir.MatmulPerfMode.DoubleRow

NCORES = 8
N = 4096
NL = N // NCORES          # 512 nodes per core
CIN = 1024
H = 1024
CO = 512
R = 10
P = 128
JT = N // P               # 32 j-tiles
ET = H // P               # 8
IT = NL // P              # 4 local-node tiles
GT = CO // P              # 4
KC = CIN // P             # 8 cin k-tiles
BN_EPS = 1e-5
HQ = H // 2               # 512: Y1 AG half width
SM = 128.0                # fp8 scale on the adjacency block
SY1 = 4.0                 # fp8 scale on Y1
SY2 = 8.0                 # fp8 scale on Y2


def build():
    """Build the SPMD Bass graph (identical on all 8 cores)."""
    nc = bacc.Bacc(None, target_bir_lowering=False, debug=False, num_devices=NCORES)

    # ---- external parameters (per-core shards / replicated) ----
    xT_d = nc.declare_dram_parameter("xT", [CIN, NL], BF16, isOutput=False)
    wmap_d = nc.declare_dram_parameter("wmap", [CIN, H], BF16, isOutput=False)
    w1_d = nc.declare_dram_parameter("w1", [H, H], BF16, isOutput=False)
    w2_d = nc.declare_dram_parameter("w2", [H, CO], BF16, isOutput=False)
    nv1T_d = nc.declare_dram_parameter("nv1T", [R, N], BF16, isOutput=False)
    nv2s_d = nc.declare_dram_parameter("nv2s", [R, NL], BF16, isOutput=False)
    bmap_d = nc.declare_dram_parameter("bmap_t", [P, ET], F32, isOutput=False)
    b1_d = nc.declare_dram_parameter("b1_t", [P, ET], F32, isOutput=False)
    s1_d = nc.declare_dram_parameter("s1_t", [P, ET], F32, isOutput=False)
    t1_d = nc.declare_dram_parameter("t1_t", [P, ET], F32, isOutput=False)
    b2_d = nc.declare_dram_parameter("b2_t", [P, GT], F32, isOutput=False)
    s2_d = nc.declare_dram_parameter("s2_t", [P, GT], F32, isOutput=False)
    t2_d = nc.declare_dram_parameter("t2_t", [P, GT], F32, isOutput=False)
    out_d = nc.declare_dram_parameter("out", [P, GT], F32, isOutput=True)

    # ---- internal DRAM: collective bounce buffers + dinv scratch ----
    rg = [list(range(NCORES))]
    rs_in = nc.dram_tensor("rs_in", [P, JT], F32)
    rs_out = nc.dram_tensor("rs_out", [NCORES * P, JT], F32, addr_space="Shared")
    y1_in = [nc.dram_tensor(f"y1_in{q}", [NL, HQ], F8) for q in range(2)]
    y1_out = [
        nc.dram_tensor(f"y1_out{q}", [N, HQ], F8, addr_space="Shared")
        for q in range(2)
    ]
    y2_in = nc.dram_tensor("y2_in", [NL, CO], F8)
    y2_out = nc.dram_tensor("y2_out", [N, CO], F8, addr_space="Shared")
    dv_dram = nc.dram_tensor("dv_dram", [NL], F32)

    cc_insts = []

    def collective(in_ap, out_ap):
        cc = nc.gpsimd.collective_compute(
            "AllGather", OP.bypass, replica_groups=rg,
            ins=[in_ap], outs=[out_ap],
        )
        cc_insts.append(cc)
        return cc

    with tile.TileContext(nc) as tc:
        with ExitStack() as ctx:
            # ---------- persistent pool ----------
            pp = ctx.enter_context(tc.tile_pool(name="persist", bufs=1))

            nv1T_sb = pp.tile([R, N], BF16)
            nv2s_sb = pp.tile([R, NL], BF16)
            nc.sync.dma_start(nv1T_sb[:], nv1T_d[:])
            nc.sync.dma_start(nv2s_sb[:], nv2s_d[:])

            rs_part = pp.tile([P, JT], F32)
            rs_all = pp.tile([P, NCORES * JT], F32)
            rowsum_sb = pp.tile([P, JT], F32)
            r_sb = pp.tile([P, JT], F32)
            r_bf = pp.tile([P, JT], BF16)
            r128_sb = pp.tile([P, JT], F32)
            degs = pp.tile([1, NL], F32)
            dinv_loc = pp.tile([1, NL], F32)
            dinvT = pp.tile([P, IT], F32)
            dinv_rep = pp.tile([P, NL], F32)

            bmap_sb = pp.tile([P, ET], F32)
            b1_sb = pp.tile([P, ET], F32)
            s1_sb = pp.tile([P, ET], F32)
            t1_sb = pp.tile([P, ET], F32)
            b2_sb = pp.tile([P, GT], F32)
            s2_sb = pp.tile([P, GT], F32)
            t2_sb = pp.tile([P, GT], F32)
            for sb, d in (
                (bmap_sb, bmap_d), (b1_sb, b1_d), (s1_sb, s1_d), (t1_sb, t1_d),
                (b2_sb, b2_d), (s2_sb, s2_d), (t2_sb, t2_d),
            ):
                nc.sync.dma_start(sb[:], d[:])

            M_sb = pp.tile([P, JT * NL], BF16)       # E = exp(relu(z)) block
            M8_sb = pp.tile([P, JT, NL], F8)         # fp8(SM * r_j * E)
            ident_sb = pp.tile([P, P], F32)
            ident8 = pp.tile([P, P], F8)             # SM * I
            diag8 = pp.tile([P, IT, NL], F8)         # self-loop rhs tiles
            make_identity(nc, ident_sb[:])
            nc.vector.tensor_scalar_mul(ident8[:], ident_sb[:], SM)
            nc.gpsimd.memset(diag8[:], 0.0)
            for it in range(IT):
                nc.vector.tensor_copy(
                    diag8[:, it, it * P:(it + 1) * P], ident8[:]
                )

            xmT_sb = pp.tile([P, ET * NL], BF16)     # relu(x wmap)^T, [f, i]
            xw1bf_sb = pp.tile([P, IT * H], BF16)    # local XW1, [i, f]
            y1q8_sb = pp.tile([P, IT, H], F8)        # fp8(SY1 dinv XW1)
            h1T_sb = pp.tile([P, ET * NL], BF16)     # conv1 out, [f, i]
            y2q8_sb = pp.tile([P, IT, CO], F8)       # fp8(SY2 dinv XW2)
            pool_part = pp.tile([P, GT], F32)
            w2_sb = pp.tile([P, KC * CO], BF16)
            for kt in range(KC):
                nc.sync.dma_start(
                    w2_sb[:, kt * CO:(kt + 1) * CO],
                    w2_d[kt * P:(kt + 1) * P, :],
                )

            with ExitStack() as actx:
                tmp_pool = actx.enter_context(tc.tile_pool(name="tmpA", bufs=4))
                psA = actx.enter_context(
                    tc.tile_pool(name="psA", bufs=3, space="PSUM")
                )
                psC = actx.enter_context(
                    tc.tile_pool(name="psC", bufs=1, space="PSUM")
                )

                # ---- feature-chain inputs: issue DMAs first, they overlap z
                pb = actx.enter_context(tc.tile_pool(name="phaseB", bufs=1))
                psB = actx.enter_context(
                    tc.tile_pool(name="psB", bufs=2, space="PSUM")
                )
                xT_sb = pb.tile([P, KC * NL], BF16)
                wm_sb = pb.tile([P, KC * H], BF16)
                w1_sb = pb.tile([P, KC * H], BF16)
                for kt in range(KC):
                    nc.sync.dma_start(
                        xT_sb[:, kt * NL:(kt + 1) * NL],
                        xT_d[kt * P:(kt + 1) * P, :],
                    )
                for kt in range(KC):
                    nc.sync.dma_start(
                        wm_sb[:, kt * H:(kt + 1) * H],
                        wmap_d[kt * P:(kt + 1) * P, :],
                    )
                for kt in range(KC):
                    nc.sync.dma_start(
                        w1_sb[:, kt * H:(kt + 1) * H],
                        w1_d[kt * P:(kt + 1) * P, :],
                    )

                # ---- z = nv1 @ nv2 column block; E = exp(relu(z)) + rowsum
                for jt in range(JT):
                    zp = psA.tile([P, NL], F32, tag="zp", name=f"zp{jt}")
                    nc.tensor.matmul(
                        zp[:],
                        nv1T_sb[:, jt * P:(jt + 1) * P],
                        nv2s_sb[:],
                        start=True, stop=True,
                    )
                    zr = tmp_pool.tile([P, NL], BF16, tag="zr", name=f"zr{jt}")
                    nc.vector.tensor_scalar_max(zr[:], zp[:], 0.0)
                    nc.scalar.activation(
                        M_sb[:, jt * NL:(jt + 1) * NL], zr[:], AF.Exp,
                        accum_out=rs_part[:, jt:jt + 1],
                    )

                # ---- AllGather softmax row-sum partials, reduce locally ----
                nc.gpsimd.dma_start(rs_in[:], rs_part[:])
                collective(rs_in[:], rs_out[:])
                nc.gpsimd.dma_start(
                    rs_all[:].rearrange("p (c t) -> p c t", t=JT),
                    rs_out.rearrange("(c p) t -> p c t", p=P),
                )
                nc.vector.tensor_add(
                    rowsum_sb[:], rs_all[:, 0:JT], rs_all[:, JT:2 * JT]
                )
                for c in range(2, NCORES):
                    nc.vector.tensor_add(
                        rowsum_sb[:], rowsum_sb[:],
                        rs_all[:, c * JT:(c + 1) * JT],
                    )
                nc.vector.reciprocal(r_sb[:], rowsum_sb[:])
                nc.vector.tensor_copy(r_bf[:], r_sb[:])
                nc.vector.tensor_scalar_mul(r128_sb[:], r_sb[:], SM)

                # ---- M8 = fp8(SM * r_j * E) (vector, overlaps Y1 gather) ----
                for jt in range(JT):
                    nc.vector.tensor_scalar_mul(
                        M8_sb[:, jt, :],
                        M_sb[:, jt * NL:(jt + 1) * NL],
                        r128_sb[:, jt:jt + 1],
                    )

                # ---- xmT = relu(wmap^T x^T + b) ----
                for et in range(ET):
                    mp = psB.tile([P, NL], F32, tag="mp", name=f"mp{et}")
                    for kt in range(KC):
                        nc.tensor.matmul(
                            mp[:],
                            wm_sb[:, kt * H + et * P: kt * H + (et + 1) * P],
                            xT_sb[:, kt * NL:(kt + 1) * NL],
                            start=(kt == 0), stop=(kt == KC - 1),
                        )
                    nc.scalar.activation(
                        xmT_sb[:, et * NL:(et + 1) * NL], mp[:], AF.Relu,
                        bias=bmap_sb[:, et:et + 1],
                    )

                # ---- XW1 = xm @ w1 (bf16 local copy; fp8 cast once dinv up)
                for q in range(2):
                    for it in range(IT):
                        wp = psB.tile([P, HQ], F32, tag="mp", name=f"wp{q}{it}")
                        for kt in range(KC):
                            nc.tensor.matmul(
                                wp[:],
                                xmT_sb[:, kt * NL + it * P: kt * NL + (it + 1) * P],
                                w1_sb[:, kt * H + q * HQ: kt * H + (q + 1) * HQ],
                                start=(kt == 0), stop=(kt == KC - 1),
                            )
                        nc.vector.tensor_copy(
                            xw1bf_sb[:, it * H + q * HQ: it * H + (q + 1) * HQ],
                            wp[:],
                        )

                # ---- degs/dinv: colsum(r_j E) + 1, local only ----
                csp = psC.tile([1, NL], F32)
                for jt in range(JT):
                    nc.tensor.matmul(
                        csp[:],
                        r_bf[:, jt:jt + 1],
                        M_sb[:, jt * NL:(jt + 1) * NL],
                        start=(jt == 0), stop=(jt == JT - 1),
                    )
                nc.scalar.activation(degs[:], csp[:], AF.Sqrt, bias=1.0)
                nc.vector.reciprocal(dinv_loc[:], degs[:])
                nc.sync.dma_start(dv_dram[:], dinv_loc[:])
                nc.sync.dma_start(
                    dinvT[:], dv_dram.rearrange("(t p) -> p t", p=P)
                )
                nc.gpsimd.dma_start(
                    dinv_rep[:], dv_dram[None, :].to_broadcast((P, NL))
                )

                # ---- Y1 = fp8(SY1 * dinv_i * XW1), write out, AllGather ----
                for it in range(IT):
                    nc.vector.tensor_scalar(
                        y1q8_sb[:, it, :],
                        xw1bf_sb[:, it * H:(it + 1) * H],
                        dinvT[:, it:it + 1], SY1,
                        op0=OP.mult, op1=OP.mult,
                    )
                for q in range(2):
                    for it in range(IT):
                        nc.scalar.dma_start(
                            y1_in[q][it * P:(it + 1) * P, :],
                            y1q8_sb[:, it, q * HQ:(q + 1) * HQ],
                        )
                for q in range(2):
                    collective(y1_in[q][:], y1_out[q][:])

            # ---------- conv passes (fp8 DoubleRow) ----------
            def conv_pass(mts, slab_pool, ps_pool, y_out_q, width,
                          off_f, yq8, tagp):
                """psum[mt] = M8^T @ Ygathered + SM * Y_self, DoubleRow fp8."""
                psums = {
                    mt: ps_pool.tile([P, NL], F32, tag=f"{tagp}{mt}",
                                     name=f"{tagp}{mt}")
                    for mt in mts
                }
                for kp in range(JT // 2):
                    slab = slab_pool.tile([P, 2, width], F8, tag=f"sl{tagp}",
                                          name=f"sl{tagp}{kp}")
                    nc.sync.dma_start(
                        slab[:, 0, :],
                        y_out_q[(2 * kp) * P:(2 * kp + 1) * P, :],
                    )
                    nc.sync.dma_start(
                        slab[:, 1, :],
                        y_out_q[(2 * kp + 1) * P:(2 * kp + 2) * P, :],
                    )
                    for mt in mts:
                        fo = mt * P - off_f
                        nc.tensor.matmul(
                            psums[mt][:],
                            slab[:, :, fo:fo + P],
                            M8_sb[:, 2 * kp:2 * kp + 2, :],
                            start=(kp == 0), stop=False,
                            perf_mode=DR,
                        )
                for tp in range(IT // 2):
                    for mt in mts:
                        nc.tensor.matmul(
                            psums[mt][:],
                            yq8[:, 2 * tp:2 * tp + 2, mt * P:(mt + 1) * P],
                            diag8[:, 2 * tp:2 * tp + 2, :],
                            start=False, stop=(tp == IT // 2 - 1),
                            perf_mode=DR,
                        )
                return psums

            def conv_epilogue(mts, psums, etmp_pool, b_sb, s_sb, t_sb, hT,
                              inv_scale, tagp, pool_out=None):
                for mt in mts:
                    ta = etmp_pool.tile([P, NL], F32, tag=f"ea{tagp}",
                                        name=f"ea{tagp}{mt}")
                    nc.vector.tensor_mul(ta[:], psums[mt][:], dinv_rep[:])
                    tb = etmp_pool.tile([P, NL], F32, tag=f"eb{tagp}",
                                        name=f"eb{tagp}{mt}")
                    nc.scalar.activation(
                        tb[:], ta[:], AF.Relu,
                        bias=b_sb[:, mt:mt + 1], scale=inv_scale,
                    )
                    nc.vector.tensor_scalar(
                        hT[:, mt * NL:(mt + 1) * NL], tb[:],
                        s_sb[:, mt:mt + 1], t_sb[:, mt:mt + 1],
                        op0=OP.mult, op1=OP.add,
                        accum_out=(
                            None if pool_out is None
                            else pool_out[:, mt:mt + 1]
                        ),
                    )

            with ExitStack() as cctx:
                slab_pool = cctx.enter_context(tc.tile_pool(name="slab1", bufs=4))
                etmp = cctx.enter_context(tc.tile_pool(name="etmp", bufs=2))
                for q in range(2):
                    with ExitStack() as cq:
                        psq = cq.enter_context(
                            tc.tile_pool(name=f"ps1{q}", bufs=1, space="PSUM")
                        )
                        mts = range(4 * q, 4 * q + 4)
                        psums = conv_pass(mts, slab_pool, psq, y1_out[q],
                                          HQ, q * HQ, y1q8_sb, f"c1{q}")
                        conv_epilogue(mts, psums, etmp, b1_sb, s1_sb,
                                      t1_sb, h1T_sb, 1.0 / (SM * SY1), f"1{q}")

                # ---- Y2 = fp8(SY2 * dinv_i * (h1 @ w2)), single AllGather --
                with ExitStack() as dctx:
                    ps2 = dctx.enter_context(
                        tc.tile_pool(name="ps2", bufs=2, space="PSUM")
                    )
                    for it in range(IT):
                        wp2 = ps2.tile([P, CO], F32, tag="wp2", name=f"wp2{it}")
                        for kt in range(ET):
                            nc.tensor.matmul(
                                wp2[:],
                                h1T_sb[:, kt * NL + it * P: kt * NL + (it + 1) * P],
                                w2_sb[:, kt * CO:(kt + 1) * CO],
                                start=(kt == 0), stop=(kt == ET - 1),
                            )
                        nc.vector.tensor_scalar(
                            y2q8_sb[:, it, :], wp2[:],
                            dinvT[:, it:it + 1], SY2,
                            op0=OP.mult, op1=OP.mult,
                        )
                        nc.scalar.dma_start(
                            y2_in[it * P:(it + 1) * P, :],
                            y2q8_sb[:, it, :],
                        )
                    collective(y2_in[:], y2_out[:])

            # ---------- conv2 (single pass) + mean-pool ----------
            with ExitStack() as ectx:
                slab2_pool = ectx.enter_context(tc.tile_pool(name="slab2", bufs=4))
                etmp2 = ectx.enter_context(tc.tile_pool(name="etmp2", bufs=2))
                ps3 = ectx.enter_context(
                    tc.tile_pool(name="ps3", bufs=1, space="PSUM")
                )
                mts = range(GT)
                psums = conv_pass(mts, slab2_pool, ps3, y2_out,
                                  CO, 0, y2q8_sb, "c2")
                conv_epilogue(mts, psums, etmp2, b2_sb, s2_sb, t2_sb,
                              h1T_sb, 1.0 / (SM * SY2), "2",
                              pool_out=pool_part)

            # per-core pooled partial out; host reduces across cores
            nc.gpsimd.dma_start(out_d[:], pool_part[:])

        # pin the CC stream order: rs, y1_h0, y1_h1, y2
        for a, b in zip(cc_insts[1:], cc_insts[:-1]):
            add_dep_helper(a.ins, b.ins, True, "cc stream order")

    nc.compile()
    return nc


_NC_CACHE = {}


def _get_nc():
    if "nc" not in _NC_CACHE:
        _NC_CACHE["nc"] = build()
    return _NC_CACHE["nc"]


def make_in_maps(inputs):
    import ml_dtypes

    f = np.float32
    bf = ml_dtypes.bfloat16
    x = np.asarray(inputs["x"], dtype=f)
    w_map = np.asarray(inputs["w_map"], dtype=f)
    w1 = np.asarray(inputs["w1"], dtype=f)
    w2 = np.asarray(inputs["w2"], dtype=f)
    nv1 = np.asarray(inputs["nv1"], dtype=f)
    nv2 = np.asarray(inputs["nv2"], dtype=f)

    def vec_t(v, nt):
        return np.ascontiguousarray(np.asarray(v, dtype=f).reshape(nt, P).T)

    s1 = (np.asarray(inputs["bn1_g"], f)
          / np.sqrt(np.asarray(inputs["bn1_v"], f) + BN_EPS))
    t1 = np.asarray(inputs["bn1_b"], f) - np.asarray(inputs["bn1_m"], f) * s1
    s2 = (np.asarray(inputs["bn2_g"], f)
          / np.sqrt(np.asarray(inputs["bn2_v"], f) + BN_EPS))
    t2 = np.asarray(inputs["bn2_b"], f) - np.asarray(inputs["bn2_m"], f) * s2

    common = {
        "wmap": np.ascontiguousarray(w_map.astype(bf)),
        "w1": np.ascontiguousarray(w1.astype(bf)),
        "w2": np.ascontiguousarray(w2.astype(bf)),
        "nv1T": np.ascontiguousarray(nv1.T.astype(bf)),
        "bmap_t": vec_t(inputs["b_map"], ET),
        "b1_t": vec_t(inputs["b1"], ET),
        "s1_t": vec_t(s1, ET),
        "t1_t": vec_t(t1, ET),
        "b2_t": vec_t(inputs["b2"], GT),
        "s2_t": vec_t(s2, GT),
        "t2_t": vec_t(t2, GT),
    }
    in_maps = []
    for c in range(NCORES):
        m = dict(common)
        m["xT"] = np.ascontiguousarray(x[c * NL:(c + 1) * NL].T.astype(bf))
        m["nv2s"] = np.ascontiguousarray(nv2[:, c * NL:(c + 1) * NL].astype(bf))
        in_maps.append(m)
    return in_maps


def finish_host(results, inputs):
    """Sum per-core pooled partials, apply mean + attention gate."""
    f = np.float32
    pooled_sum = np.zeros(CO, f)
    for res in results:
        arr = np.asarray(res["out"], dtype=f)      # [P, GT], g = t*P + p
        pooled_sum += arr.T.reshape(-1)
    pooled = pooled_sum / N
    w_attn = np.asarray(inputs["w_attn"], f).reshape(-1)
    b_attn = np.asarray(inputs["b_attn"], f).reshape(-1)[0]
    z = float(pooled @ w_attn + b_attn)
    attn = 1.0 / (1.0 + np.exp(-z))
    return (pooled * attn)[None, :].astype(f)


def run(inputs, trace=False, tmpdir=None):
    nc = _get_nc()
    in_maps = make_in_maps(inputs)
    res = run_bass_kernel_spmd(
        nc, in_maps, core_ids=list(range(NCORES)), trace=trace, tmpdir=tmpdir
    )
    out = finish_host(res.results, inputs)
    return out, res


def kernel(**inputs):
    out, _ = run(inputs)
    return out


# revision 9
# speedup vs baseline: 1.3075x; 1.1740x over previous
"""Distributed Trainium2 Bass kernel for AdaptiveGCN (N=4096, CIN=1024, H=1024, COUT=512, R=10).

Sharding: node dimension split across 8 NeuronCores (512 nodes/core).
Each core owns a 512-column block of the dense adaptive adjacency and the
matching 512 output rows.

v2 layout of the math (per core):
  E        = exp(relu(nv1 @ nv2))            column block, bf16
  r_j      = 1 / rowsum(E)                   (rowsum partials AllGathered)
  M8       = fp8(128 * r_j * E)              adjacency block for fp8 DoubleRow
  deg_i    = colsum(r_j E) + 1  (local),  dinv_i = deg_i^-1/2   (local, NO gather)
  Y1       = fp8(4 * dinv_i * (xm @ w1))     scaled BEFORE the AllGather, fp8 wire
  conv1    = relu((M8^T @ Y1 + 128*Y1_self) * dinv_i / 512 + b1) -> bn1 -> h1
  Y2       = fp8(8 * dinv_i * (h1 @ w2))     single fp8 AllGather
  conv2    = same shape, then mean-pool partials returned per core.

Collectives (CC stream order): rowsum partials (16KB f32), Y1 in two fp8
halves (256KB each), Y2 in one fp8 shot (256KB). The dinv gather of the
baseline is gone (dinv is folded into Y before the gather).

kernel(**inputs) takes the FULL unsharded inputs (same keys as
reference.setup_inputs()) and returns the FULL [1, 512] float32 output.
"""

import os
import sys
from contextlib import ExitStack

import numpy as np

for _p in ("/opt/trn_rl_repo", "/root/.axon_site/_ro/trn_rl_repo"):
    if os.path.isdir(_p) and _p not in sys.path:
        sys.path.insert(0, _p)

import concourse.bass as bass
import concourse.bacc as bacc
import concourse.tile as tile
from concourse import mybir
from concourse.bass_utils import run_bass_kernel_spmd
from concourse.masks import make_identity
from concourse.tile_rust import add_dep_helper

F32 = mybir.dt.float32
BF16 = mybir.dt.bfloat16
F8 = mybir.dt.float8e4
AF = mybir.ActivationFunctionType
OP = mybir.AluOpType
AX = mybir.AxisListType
DR = mybir.MatmulPerfMode.DoubleRow

NCORES = 8
N = 4096
NL = N // NCORES          # 512 nodes per core
CIN = 1024
H = 1024
CO = 512
R = 10
P = 128
JT = N // P               # 32 j-tiles
ET = H // P               # 8
IT = NL // P              # 4 local-node tiles
GT = CO // P              # 4
KC = CIN // P             # 8 cin k-tiles
BN_EPS = 1e-5
HQ = H // 2               # 512: Y1 AG half width
SM = 128.0                # fp8 scale on the adjacency block
SY1 = 4.0                 # fp8 scale on Y1
SY2 = 8.0                 # fp8 scale on Y2


def build():
    """Build the SPMD Bass graph (identical on all 8 cores)."""
    nc = bacc.Bacc(None, target_bir_lowering=False, debug=False, num_devices=NCORES)

    # ---- external parameters (per-core shards / replicated) ----
    xT_d = nc.declare_dram_parameter("xT", [CIN, NL], BF16, isOutput=False)
    wmap_d = nc.declare_dram_parameter("wmap", [CIN, H], BF16, isOutput=False)
    w1_d = nc.declare_dram_parameter("w1", [H, H], BF16, isOutput=False)
    w2_d = nc.declare_dram_parameter("w2", [H, CO], BF16, isOutput=False)
    nv1T_d = nc.declare_dram_parameter("nv1T", [R, N], BF16, isOutput=False)
    nv2s_d = nc.declare_dram_parameter("nv2s", [R, NL], BF16, isOutput=False)
    bmap_d = nc.declare_dram_parameter("bmap_t", [P, ET], F32, isOutput=False)
    b1_d = nc.declare_dram_parameter("b1_t", [P, ET], F32, isOutput=False)
    s1_d = nc.declare_dram_parameter("s1_t", [P, ET], F32, isOutput=False)
    t1_d = nc.declare_dram_parameter("t1_t", [P, ET], F32, isOutput=False)
    b2_d = nc.declare_dram_parameter("b2_t", [P, GT], F32, isOutput=False)
    s2_d = nc.declare_dram_parameter("s2_t", [P, GT], F32, isOutput=False)
    t2_d = nc.declare_dram_parameter("t2_t", [P, GT], F32, isOutput=False)
    out_d = nc.declare_dram_parameter("out", [P, GT], F32, isOutput=True)

    # ---- internal DRAM: collective bounce buffers + dinv scratch ----
    rg = [list(range(NCORES))]
    rs_in = nc.dram_tensor("rs_in", [P, JT], F32)
    rs_out = nc.dram_tensor("rs_out", [NCORES * P, JT], F32, addr_space="Shared")
    y1_in = [nc.dram_tensor(f"y1_in{q}", [NL, HQ], F8) for q in range(2)]
    y1_out = [
        nc.dram_tensor(f"y1_out{q}", [N, HQ], F8, addr_space="Shared")
        for q in range(2)
    ]
    y2_in = nc.dram_tensor("y2_in", [NL, CO], F8)
    y2_out = nc.dram_tensor("y2_out", [N, CO], F8, addr_space="Shared")
    dv_dram = nc.dram_tensor("dv_dram", [NL], F32)

    cc_insts = []

    def collective(in_ap, out_ap):
        cc = nc.gpsimd.collective_compute(
            "AllGather", OP.bypass, replica_groups=rg,
            ins=[in_ap], outs=[out_ap],
        )
        cc_insts.append(cc)
        return cc

    with tile.TileContext(nc) as tc:
        with ExitStack() as ctx:
            # ---------- persistent pool ----------
            pp = ctx.enter_context(tc.tile_pool(name="persist", bufs=1))

            nv1T_sb = pp.tile([R, N], BF16)
            nv2s_sb = pp.tile([R, NL], BF16)
            nc.sync.dma_start(nv1T_sb[:], nv1T_d[:])
            nc.sync.dma_start(nv2s_sb[:], nv2s_d[:])

            xT_sb = pp.tile([P, KC * NL], BF16)
            wm_sb = pp.tile([P, KC * H], BF16)
            w1_sb = pp.tile([P, KC * H], BF16)
            nc.sync.dma_start(
                xT_sb[:].rearrange("p (t w) -> p t w", w=NL),
                xT_d.rearrange("(t p) w -> p t w", p=P),
            )
            nc.sync.dma_start(
                wm_sb[:].rearrange("p (t w) -> p t w", w=H),
                wmap_d.rearrange("(t p) w -> p t w", p=P),
            )
            nc.sync.dma_start(
                w1_sb[:].rearrange("p (t w) -> p t w", w=H),
                w1_d.rearrange("(t p) w -> p t w", p=P),
            )

            rs_part = pp.tile([P, JT], F32)
            rs_all = pp.tile([P, NCORES * JT], F32)
            rowsum_sb = pp.tile([P, JT], F32)
            r_sb = pp.tile([P, JT], F32)
            r_bf = pp.tile([P, JT], BF16)
            r128_sb = pp.tile([P, JT], F32)
            degs = pp.tile([1, NL], F32)
            dinv_loc = pp.tile([1, NL], F32)
            dinvT = pp.tile([P, IT], F32)
            dinv_rep = pp.tile([P, NL], F32)

            bmap_sb = pp.tile([P, ET], F32)
            b1_sb = pp.tile([P, ET], F32)
            s1_sb = pp.tile([P, ET], F32)
            t1_sb = pp.tile([P, ET], F32)
            b2_sb = pp.tile([P, GT], F32)
            s2_sb = pp.tile([P, GT], F32)
            t2_sb = pp.tile([P, GT], F32)
            for sb, d in (
                (bmap_sb, bmap_d), (b1_sb, b1_d), (s1_sb, s1_d), (t1_sb, t1_d),
                (b2_sb, b2_d), (s2_sb, s2_d), (t2_sb, t2_d),
            ):
                nc.sync.dma_start(sb[:], d[:])

            M_sb = pp.tile([P, JT * NL], BF16)       # E = exp(relu(z)) block
            M8_sb = pp.tile([P, JT, NL], F8)         # fp8(SM * r_j * E)
            ident_sb = pp.tile([P, P], F32)
            ident8 = pp.tile([P, P], F8)             # SM * I
            diag8 = pp.tile([P, IT, NL], F8)         # self-loop rhs tiles
            make_identity(nc, ident_sb[:])
            nc.vector.tensor_scalar_mul(ident8[:], ident_sb[:], SM)
            nc.gpsimd.memset(diag8[:], 0.0)
            for it in range(IT):
                nc.vector.tensor_copy(
                    diag8[:, it, it * P:(it + 1) * P], ident8[:]
                )

            xmT_sb = pp.tile([P, ET * NL], BF16)     # relu(x wmap)^T, [f, i]
            xw1bf_sb = pp.tile([P, IT * H], BF16)    # local XW1, [i, f]
            y1q8_sb = pp.tile([P, IT, H], F8)        # fp8(SY1 dinv XW1)
            h1T_sb = pp.tile([P, ET * NL], BF16)     # conv1 out, [f, i]
            y2q8_sb = pp.tile([P, IT, CO], F8)       # fp8(SY2 dinv XW2)
            pool_part = pp.tile([P, GT], F32)
            w2_sb = pp.tile([P, KC * CO], BF16)
            nc.sync.dma_start(
                w2_sb[:].rearrange("p (t w) -> p t w", w=CO),
                w2_d.rearrange("(t p) w -> p t w", p=P),
            )

            with ExitStack() as actx:
                tmp_pool = actx.enter_context(tc.tile_pool(name="tmpA", bufs=4))
                psA = actx.enter_context(
                    tc.tile_pool(name="psA", bufs=3, space="PSUM")
                )
                psC = actx.enter_context(
                    tc.tile_pool(name="psC", bufs=1, space="PSUM")
                )

                psB = actx.enter_context(
                    tc.tile_pool(name="psB", bufs=2, space="PSUM")
                )

                # ---- z = nv1 @ nv2 column block; E = exp(relu(z)) + rowsum.
                # exp(relu(z)) == max(exp(z), 1): scalar exp straight off
                # PSUM, vector does the max at 2x bf16 rate with the row-sum
                # accumulated in the same pass.
                for jt in range(JT):
                    zp = psA.tile([P, NL], F32, tag="zp", name=f"zp{jt}")
                    nc.tensor.matmul(
                        zp[:],
                        nv1T_sb[:, jt * P:(jt + 1) * P],
                        nv2s_sb[:],
                        start=True, stop=True,
                    )
                    ez = tmp_pool.tile([P, NL], BF16, tag="ez", name=f"ez{jt}")
                    nc.scalar.activation(ez[:], zp[:], AF.Exp)
                    # with accum_out, op1 is the REDUCE op: dst = max(in, 1),
                    # accum = rowsum(dst) + scalar2
                    nc.vector.tensor_scalar(
                        M_sb[:, jt * NL:(jt + 1) * NL], ez[:], 1.0, 0.0,
                        op0=OP.max, op1=OP.add,
                        accum_out=rs_part[:, jt:jt + 1],
                    )

                # ---- AllGather softmax row-sum partials, reduce locally ----
                nc.gpsimd.dma_start(rs_in[:], rs_part[:])
                collective(rs_in[:], rs_out[:])
                nc.gpsimd.dma_start(
                    rs_all[:].rearrange("p (c t) -> p c t", t=JT),
                    rs_out.rearrange("(c p) t -> p c t", p=P),
                )
                nc.vector.tensor_add(
                    rowsum_sb[:], rs_all[:, 0:JT], rs_all[:, JT:2 * JT]
                )
                for c in range(2, NCORES):
                    nc.vector.tensor_add(
                        rowsum_sb[:], rowsum_sb[:],
                        rs_all[:, c * JT:(c + 1) * JT],
                    )
                nc.vector.reciprocal(r_sb[:], rowsum_sb[:])
                nc.vector.tensor_copy(r_bf[:], r_sb[:])
                nc.vector.tensor_scalar_mul(r128_sb[:], r_sb[:], SM)

                # ---- M8 = fp8(SM * r_j * E) (vector, overlaps Y1 gather) ----
                for jt in range(JT):
                    nc.vector.tensor_scalar_mul(
                        M8_sb[:, jt, :],
                        M_sb[:, jt * NL:(jt + 1) * NL],
                        r128_sb[:, jt:jt + 1],
                    )

                # ---- xmT = relu(wmap^T x^T + b) ----
                for et in range(ET):
                    mp = psB.tile([P, NL], F32, tag="mp", name=f"mp{et}")
                    for kt in range(KC):
                        nc.tensor.matmul(
                            mp[:],
                            wm_sb[:, kt * H + et * P: kt * H + (et + 1) * P],
                            xT_sb[:, kt * NL:(kt + 1) * NL],
                            start=(kt == 0), stop=(kt == KC - 1),
                        )
                    nc.scalar.activation(
                        xmT_sb[:, et * NL:(et + 1) * NL], mp[:], AF.Relu,
                        bias=bmap_sb[:, et:et + 1],
                    )

                # ---- XW1 = xm @ w1 (bf16 local copy; fp8 cast once dinv up)
                for q in range(2):
                    for it in range(IT):
                        wp = psB.tile([P, HQ], F32, tag="mp", name=f"wp{q}{it}")
                        for kt in range(KC):
                            nc.tensor.matmul(
                                wp[:],
                                xmT_sb[:, kt * NL + it * P: kt * NL + (it + 1) * P],
                                w1_sb[:, kt * H + q * HQ: kt * H + (q + 1) * HQ],
                                start=(kt == 0), stop=(kt == KC - 1),
                            )
                        nc.vector.tensor_copy(
                            xw1bf_sb[:, it * H + q * HQ: it * H + (q + 1) * HQ],
                            wp[:],
                        )

                # ---- degs/dinv: colsum(r_j E) + 1, local only ----
                csp = psC.tile([1, NL], F32)
                for jt in range(JT):
                    nc.tensor.matmul(
                        csp[:],
                        r_bf[:, jt:jt + 1],
                        M_sb[:, jt * NL:(jt + 1) * NL],
                        start=(jt == 0), stop=(jt == JT - 1),
                    )
                nc.scalar.activation(degs[:], csp[:], AF.Sqrt, bias=1.0)
                nc.vector.reciprocal(dinv_loc[:], degs[:])
                nc.sync.dma_start(dv_dram[:], dinv_loc[:])
                nc.sync.dma_start(
                    dinvT[:], dv_dram.rearrange("(t p) -> p t", p=P)
                )
                nc.gpsimd.dma_start(
                    dinv_rep[:], dv_dram[None, :].to_broadcast((P, NL))
                )

                # ---- Y1 = fp8(SY1 * dinv_i * XW1), write out, AllGather ----
                for it in range(IT):
                    nc.vector.tensor_scalar(
                        y1q8_sb[:, it, :],
                        xw1bf_sb[:, it * H:(it + 1) * H],
                        dinvT[:, it:it + 1], SY1,
                        op0=OP.mult, op1=OP.mult,
                    )
                for q in range(2):
                    nc.scalar.dma_start(
                        y1_in[q].rearrange("(t p) w -> p t w", p=P),
                        y1q8_sb[:, :, q * HQ:(q + 1) * HQ],
                    )
                for q in range(2):
                    collective(y1_in[q][:], y1_out[q][:])

            # ---------- conv passes (fp8 DoubleRow) ----------
            def conv_pass(mts, slab_pool, ps_pool, y_out_q, width,
                          off_f, yq8, tagp):
                """psum[mt] = M8^T @ Ygathered + SM * Y_self, DoubleRow fp8."""
                psums = {
                    mt: ps_pool.tile([P, NL], F32, tag=f"{tagp}{mt}",
                                     name=f"{tagp}{mt}")
                    for mt in mts
                }
                SK = 8                      # k-tiles per slab super-load
                for s in range(JT // SK):
                    slab = slab_pool.tile([P, SK, width], F8, tag=f"sl{tagp}",
                                          name=f"sl{tagp}{s}")
                    nc.sync.dma_start(
                        slab[:],
                        y_out_q[s * SK * P:(s + 1) * SK * P, :]
                        .rearrange("(t p) w -> p t w", p=P),
                    )
                    for kp in range(SK // 2):
                        for mt in mts:
                            fo = mt * P - off_f
                            nc.tensor.matmul(
                                psums[mt][:],
                                slab[:, 2 * kp:2 * kp + 2, fo:fo + P],
                                M8_sb[:, s * SK + 2 * kp:s * SK + 2 * kp + 2, :],
                                start=(s == 0 and kp == 0), stop=False,
                                perf_mode=DR,
                            )
                # mt-major so psums stop one by one and epilogues overlap
                for mt in mts:
                    for tp in range(IT // 2):
                        nc.tensor.matmul(
                            psums[mt][:],
                            yq8[:, 2 * tp:2 * tp + 2, mt * P:(mt + 1) * P],
                            diag8[:, 2 * tp:2 * tp + 2, :],
                            start=False, stop=(tp == IT // 2 - 1),
                            perf_mode=DR,
                        )
                return psums

            def conv_epilogue(mts, psums, etmp_pool, b_sb, s_sb, t_sb, hT,
                              inv_scale, tagp, pool_out=None):
                for mt in mts:
                    ta = etmp_pool.tile([P, NL], F32, tag=f"ea{tagp}",
                                        name=f"ea{tagp}{mt}")
                    nc.vector.tensor_mul(ta[:], psums[mt][:], dinv_rep[:])
                    tb = etmp_pool.tile([P, NL], F32, tag=f"eb{tagp}",
                                        name=f"eb{tagp}{mt}")
                    nc.scalar.activation(
                        tb[:], ta[:], AF.Relu,
                        bias=b_sb[:, mt:mt + 1], scale=inv_scale,
                    )
                    nc.vector.tensor_scalar(
                        hT[:, mt * NL:(mt + 1) * NL], tb[:],
                        s_sb[:, mt:mt + 1], t_sb[:, mt:mt + 1],
                        op0=OP.mult, op1=OP.add,
                        accum_out=(
                            None if pool_out is None
                            else pool_out[:, mt:mt + 1]
                        ),
                    )

            with ExitStack() as cctx:
                slab_pool = cctx.enter_context(tc.tile_pool(name="slab1", bufs=4))
                etmp = cctx.enter_context(tc.tile_pool(name="etmp", bufs=2))
                for q in range(2):
                    with ExitStack() as cq:
                        psq = cq.enter_context(
                            tc.tile_pool(name=f"ps1{q}", bufs=1, space="PSUM")
                        )
                        mts = range(4 * q, 4 * q + 4)
                        psums = conv_pass(mts, slab_pool, psq, y1_out[q],
                                          HQ, q * HQ, y1q8_sb, f"c1{q}")
                        conv_epilogue(mts, psums, etmp, b1_sb, s1_sb,
                                      t1_sb, h1T_sb, 1.0 / (SM * SY1), f"1{q}")

                # ---- Y2 = fp8(SY2 * dinv_i * (h1 @ w2)), single AllGather --
                with ExitStack() as dctx:
                    ps2 = dctx.enter_context(
                        tc.tile_pool(name="ps2", bufs=2, space="PSUM")
                    )
                    for it in range(IT):
                        wp2 = ps2.tile([P, CO], F32, tag="wp2", name=f"wp2{it}")
                        for kt in range(ET):
                            nc.tensor.matmul(
                                wp2[:],
                                h1T_sb[:, kt * NL + it * P: kt * NL + (it + 1) * P],
                                w2_sb[:, kt * CO:(kt + 1) * CO],
                                start=(kt == 0), stop=(kt == ET - 1),
                            )
                        nc.vector.tensor_scalar(
                            y2q8_sb[:, it, :], wp2[:],
                            dinvT[:, it:it + 1], SY2,
                            op0=OP.mult, op1=OP.mult,
                        )
                        nc.scalar.dma_start(
                            y2_in[it * P:(it + 1) * P, :],
                            y2q8_sb[:, it, :],
                        )
                    collective(y2_in[:], y2_out[:])

            # ---------- conv2 (single pass) + mean-pool ----------
            with ExitStack() as ectx:
                slab2_pool = ectx.enter_context(tc.tile_pool(name="slab2", bufs=4))
                etmp2 = ectx.enter_context(tc.tile_pool(name="etmp2", bufs=2))
                ps3 = ectx.enter_context(
                    tc.tile_pool(name="ps3", bufs=1, space="PSUM")
                )
                mts = range(GT)
                psums = conv_pass(mts, slab2_pool, ps3, y2_out,
                                  CO, 0, y2q8_sb, "c2")
                conv_epilogue(mts, psums, etmp2, b2_sb, s2_sb, t2_sb,
                              h1T_sb, 1.0 / (SM * SY2), "2",
                              pool_out=pool_part)

            # per-core pooled partial out; host reduces across cores
            nc.gpsimd.dma_start(out_d[:], pool_part[:])

        # pin the CC stream order: rs, y1_h0, y1_h1, y2
        for a, b in zip(cc_insts[1:], cc_insts[:-1]):
            add_dep_helper(a.ins, b.ins, True, "cc stream order")

    nc.compile()
    return nc


_NC_CACHE = {}


def _get_nc():
    if "nc" not in _NC_CACHE:
        _NC_CACHE["nc"] = build()
    return _NC_CACHE["nc"]


def make_in_maps(inputs):
    import ml_dtypes

    f = np.float32
    bf = ml_dtypes.bfloat16
    x = np.asarray(inputs["x"], dtype=f)
    w_map = np.asarray(inputs["w_map"], dtype=f)
    w1 = np.asarray(inputs["w1"], dtype=f)
    w2 = np.asarray(inputs["w2"], dtype=f)
    nv1 = np.asarray(inputs["nv1"], dtype=f)
    nv2 = np.asarray(inputs["nv2"], dtype=f)

    def vec_t(v, nt):
        return np.ascontiguousarray(np.asarray(v, dtype=f).reshape(nt, P).T)

    s1 = (np.asarray(inputs["bn1_g"], f)
          / np.sqrt(np.asarray(inputs["bn1_v"], f) + BN_EPS))
    t1 = np.asarray(inputs["bn1_b"], f) - np.asarray(inputs["bn1_m"], f) * s1
    s2 = (np.asarray(inputs["bn2_g"], f)
          / np.sqrt(np.asarray(inputs["bn2_v"], f) + BN_EPS))
    t2 = np.asarray(inputs["bn2_b"], f) - np.asarray(inputs["bn2_m"], f) * s2

    common = {
        "wmap": np.ascontiguousarray(w_map.astype(bf)),
        "w1": np.ascontiguousarray(w1.astype(bf)),
        "w2": np.ascontiguousarray(w2.astype(bf)),
        "nv1T": np.ascontiguousarray(nv1.T.astype(bf)),
        "bmap_t": vec_t(inputs["b_map"], ET),
        "b1_t": vec_t(inputs["b1"], ET),
        "s1_t": vec_t(s1, ET),
        "t1_t": vec_t(t1, ET),
        "b2_t": vec_t(inputs["b2"], GT),
        "s2_t": vec_t(s2, GT),
        "t2_t": vec_t(t2, GT),
    }
    in_maps = []
    for c in range(NCORES):
        m = dict(common)
        m["xT"] = np.ascontiguousarray(x[c * NL:(c + 1) * NL].T.astype(bf))
        m["nv2s"] = np.ascontiguousarray(nv2[:, c * NL:(c + 1) * NL].astype(bf))
        in_maps.append(m)
    return in_maps


def finish_host(results, inputs):
    """Sum per-core pooled partials, apply mean + attention gate."""
    f = np.float32
    pooled_sum = np.zeros(CO, f)
    for res in results:
        arr = np.asarray(res["out"], dtype=f)      # [P, GT], g = t*P + p
        pooled_sum += arr.T.reshape(-1)
    pooled = pooled_sum / N
    w_attn = np.asarray(inputs["w_attn"], f).reshape(-1)
    b_attn = np.asarray(inputs["b_attn"], f).reshape(-1)[0]
    z = float(pooled @ w_attn + b_attn)
    attn = 1.0 / (1.0 + np.exp(-z))
    return (pooled * attn)[None, :].astype(f)


def run(inputs, trace=False, tmpdir=None):
    nc = _get_nc()
    in_maps = make_in_maps(inputs)
    res = run_bass_kernel_spmd(
        nc, in_maps, core_ids=list(range(NCORES)), trace=trace, tmpdir=tmpdir
    )
    out = finish_host(res.results, inputs)
    return out, res


def kernel(**inputs):
    out, _ = run(inputs)
    return out


# revision 14
# speedup vs baseline: 1.3475x; 1.0306x over previous
"""Distributed Trainium2 Bass kernel for AdaptiveGCN (N=4096, CIN=1024, H=1024, COUT=512, R=10).

Sharding: node dimension split across 8 NeuronCores (512 nodes/core).
Each core owns a 512-column block of the dense adaptive adjacency and the
matching 512 output rows.

v2 layout of the math (per core):
  E        = exp(relu(nv1 @ nv2))            column block, bf16
  r_j      = 1 / rowsum(E)                   (rowsum partials AllGathered)
  M8       = fp8(128 * r_j * E)              adjacency block for fp8 DoubleRow
  deg_i    = colsum(r_j E) + 1  (local),  dinv_i = deg_i^-1/2   (local, NO gather)
  Y1       = fp8(4 * dinv_i * (xm @ w1))     scaled BEFORE the AllGather, fp8 wire
  conv1    = relu((M8^T @ Y1 + 128*Y1_self) * dinv_i / 512 + b1) -> bn1 -> h1
  Y2       = fp8(8 * dinv_i * (h1 @ w2))     single fp8 AllGather
  conv2    = same shape, then mean-pool partials returned per core.

Collectives (CC stream order): rowsum partials (16KB f32), Y1 in two fp8
halves (256KB each), Y2 in one fp8 shot (256KB). The dinv gather of the
baseline is gone (dinv is folded into Y before the gather).

kernel(**inputs) takes the FULL unsharded inputs (same keys as
reference.setup_inputs()) and returns the FULL [1, 512] float32 output.
"""

import os
import sys
from contextlib import ExitStack

import numpy as np

for _p in ("/opt/trn_rl_repo", "/root/.axon_site/_ro/trn_rl_repo"):
    if os.path.isdir(_p) and _p not in sys.path:
        sys.path.insert(0, _p)

import concourse.bass as bass
import concourse.bacc as bacc
import concourse.tile as tile
from concourse import mybir
from concourse.bass_utils import run_bass_kernel_spmd
from concourse.masks import make_identity
from concourse.tile_rust import add_dep_helper

F32 = mybir.dt.float32
BF16 = mybir.dt.bfloat16
F8 = mybir.dt.float8e4
AF = mybir.ActivationFunctionType
OP = mybir.AluOpType
AX = mybir.AxisListType
DR = mybir.MatmulPerfMode.DoubleRow

NCORES = 8
N = 4096
NL = N // NCORES          # 512 nodes per core
CIN = 1024
H = 1024
CO = 512
R = 10
P = 128
JT = N // P               # 32 j-tiles
ET = H // P               # 8
IT = NL // P              # 4 local-node tiles
GT = CO // P              # 4
KC = CIN // P             # 8 cin k-tiles
BN_EPS = 1e-5
HQ = H // 2               # 512: Y1 AG half width
SM = 128.0                # fp8 scale on the adjacency block
SY1 = 4.0                 # fp8 scale on Y1
SY2 = 8.0                 # fp8 scale on Y2


def build():
    """Build the SPMD Bass graph (identical on all 8 cores)."""
    nc = bacc.Bacc(None, target_bir_lowering=False, debug=False, num_devices=NCORES)

    # ---- external parameters (per-core shards / replicated) ----
    xT_d = nc.declare_dram_parameter("xT", [CIN, NL], BF16, isOutput=False)
    wmap_d = nc.declare_dram_parameter("wmap", [CIN, H], BF16, isOutput=False)
    w1_d = nc.declare_dram_parameter("w1", [H, H], BF16, isOutput=False)
    w2_d = nc.declare_dram_parameter("w2", [H, CO], BF16, isOutput=False)
    nv1T_d = nc.declare_dram_parameter("nv1T", [R, N], BF16, isOutput=False)
    nv2s_d = nc.declare_dram_parameter("nv2s", [R, NL], BF16, isOutput=False)
    bmap_d = nc.declare_dram_parameter("bmap_t", [P, ET], F32, isOutput=False)
    b1_d = nc.declare_dram_parameter("b1_t", [P, ET], F32, isOutput=False)
    s1_d = nc.declare_dram_parameter("s1_t", [P, ET], F32, isOutput=False)
    t1_d = nc.declare_dram_parameter("t1_t", [P, ET], F32, isOutput=False)
    b2_d = nc.declare_dram_parameter("b2_t", [P, GT], F32, isOutput=False)
    s2_d = nc.declare_dram_parameter("s2_t", [P, GT], F32, isOutput=False)
    t2_d = nc.declare_dram_parameter("t2_t", [P, GT], F32, isOutput=False)
    out_d = nc.declare_dram_parameter("out", [P, GT], F32, isOutput=True)

    # ---- internal DRAM: collective bounce buffers + dinv scratch ----
    rg = [list(range(NCORES))]
    rs_in = nc.dram_tensor("rs_in", [P, JT], F32)
    rs_out = nc.dram_tensor("rs_out", [P, JT], F32, addr_space="Shared")
    y1_in = [nc.dram_tensor(f"y1_in{q}", [NL, HQ], F8) for q in range(2)]
    y1_out = [
        nc.dram_tensor(f"y1_out{q}", [N, HQ], F8, addr_space="Shared")
        for q in range(2)
    ]
    y2_in = nc.dram_tensor("y2_in", [NL, CO], F8)
    y2_out = nc.dram_tensor("y2_out", [N, CO], F8, addr_space="Shared")
    dv_dram = nc.dram_tensor("dv_dram", [NL], F32)

    cc_insts = []

    def collective(in_ap, out_ap, kind="AllGather", op=OP.bypass):
        cc = nc.gpsimd.collective_compute(
            kind, op, replica_groups=rg,
            ins=[in_ap], outs=[out_ap],
        )
        cc_insts.append(cc)
        return cc

    with tile.TileContext(nc) as tc:
        with ExitStack() as ctx:
            # ---------- persistent pool ----------
            pp = ctx.enter_context(tc.tile_pool(name="persist", bufs=1))

            nv1T_sb = pp.tile([R, N], BF16)
            nv2s_sb = pp.tile([R, NL], BF16)
            nc.sync.dma_start(nv1T_sb[:], nv1T_d[:])
            nc.sync.dma_start(nv2s_sb[:], nv2s_d[:])

            xT_sb = pp.tile([P, KC * NL], BF16)
            wm_sb = pp.tile([P, KC * H], BF16)
            w1_sb = pp.tile([P, KC * H], BF16)
            nc.sync.dma_start(
                xT_sb[:].rearrange("p (t w) -> p t w", w=NL),
                xT_d.rearrange("(t p) w -> p t w", p=P),
            )
            nc.sync.dma_start(
                wm_sb[:].rearrange("p (t w) -> p t w", w=H),
                wmap_d.rearrange("(t p) w -> p t w", p=P),
            )
            nc.sync.dma_start(
                w1_sb[:].rearrange("p (t w) -> p t w", w=H),
                w1_d.rearrange("(t p) w -> p t w", p=P),
            )

            rs_part = pp.tile([P, JT], F32)
            rowsum_sb = pp.tile([P, JT], F32)
            r_sb = pp.tile([P, JT], F32)
            r_bf = pp.tile([P, JT], BF16)
            r128_sb = pp.tile([P, JT], F32)
            sq_loc = pp.tile([1, NL], F32)           # sqrt(deg)
            sqT = pp.tile([P, IT], F32)
            sq_rep = pp.tile([P, NL], F32)
            dinvT = pp.tile([P, IT], F32)
            dinv_rep = pp.tile([P, NL], F32)

            bmap_sb = pp.tile([P, ET], F32)
            b1_sb = pp.tile([P, ET], F32)
            s1_sb = pp.tile([P, ET], F32)
            t1_sb = pp.tile([P, ET], F32)
            b2_sb = pp.tile([P, GT], F32)
            s2_sb = pp.tile([P, GT], F32)
            t2_sb = pp.tile([P, GT], F32)
            for sb, d in (
                (bmap_sb, bmap_d), (b1_sb, b1_d), (s1_sb, s1_d), (t1_sb, t1_d),
                (b2_sb, b2_d), (s2_sb, s2_d), (t2_sb, t2_d),
            ):
                nc.sync.dma_start(sb[:], d[:])

            M_sb = pp.tile([P, JT * NL], BF16)       # E = exp(relu(z)) block
            M8_sb = pp.tile([P, JT, NL], F8)         # fp8(SM * r_j * E)
            ident_sb = pp.tile([P, P], F32)
            ident8 = pp.tile([P, P], F8)             # SM * I
            diag8 = pp.tile([P, IT, NL], F8)         # self-loop rhs tiles
            make_identity(nc, ident_sb[:])
            nc.vector.tensor_scalar_mul(ident8[:], ident_sb[:], SM)
            nc.gpsimd.memset(diag8[:], 0.0)
            for it in range(IT):
                nc.vector.tensor_copy(
                    diag8[:, it, it * P:(it + 1) * P], ident8[:]
                )

            xmT_sb = pp.tile([P, ET * NL], BF16)     # relu(x wmap)^T, [f, i]
            xw1bf_sb = pp.tile([P, IT * H], BF16)    # local XW1, [i, f]
            y1q8_sb = pp.tile([P, IT, H], F8)        # fp8(SY1 dinv XW1)
            h1T_sb = pp.tile([P, ET * NL], BF16)     # conv1 out, [f, i]
            y2q8_sb = pp.tile([P, IT, CO], F8)       # fp8(SY2 dinv XW2)
            pool_part = pp.tile([P, GT], F32)
            w2_sb = pp.tile([P, KC * CO], BF16)
            nc.sync.dma_start(
                w2_sb[:].rearrange("p (t w) -> p t w", w=CO),
                w2_d.rearrange("(t p) w -> p t w", p=P),
            )

            with ExitStack() as actx:
                tmp_pool = actx.enter_context(tc.tile_pool(name="tmpA", bufs=4))
                psA = actx.enter_context(
                    tc.tile_pool(name="psA", bufs=3, space="PSUM")
                )
                psC = actx.enter_context(
                    tc.tile_pool(name="psC", bufs=1, space="PSUM")
                )

                psB = actx.enter_context(
                    tc.tile_pool(name="psB", bufs=2, space="PSUM")
                )

                # ---- z = nv1 @ nv2 column block; E = exp(relu(z)) + rowsum.
                # exp(relu(z)) == max(exp(z), 1): scalar exp straight off
                # PSUM, vector does the max at 2x bf16 rate with the row-sum
                # accumulated in the same pass.
                for jt in range(JT):
                    zp = psA.tile([P, NL], F32, tag="zp", name=f"zp{jt}")
                    nc.tensor.matmul(
                        zp[:],
                        nv1T_sb[:, jt * P:(jt + 1) * P],
                        nv2s_sb[:],
                        start=True, stop=True,
                    )
                    ez = tmp_pool.tile([P, NL], BF16, tag="ez", name=f"ez{jt}")
                    nc.scalar.activation(ez[:], zp[:], AF.Exp)
                    # with accum_out, op1 is the REDUCE op: dst = max(in, 1),
                    # accum = rowsum(dst) + scalar2
                    nc.vector.tensor_scalar(
                        M_sb[:, jt * NL:(jt + 1) * NL], ez[:], 1.0, 0.0,
                        op0=OP.max, op1=OP.add,
                        accum_out=rs_part[:, jt:jt + 1],
                    )

                # ---- AllReduce softmax row-sum partials (fabric adds) ----
                nc.gpsimd.dma_start(rs_in[:], rs_part[:])
                collective(rs_in[:], rs_out[:], kind="AllReduce", op=OP.add)
                nc.gpsimd.dma_start(rowsum_sb[:], rs_out[:])
                nc.vector.reciprocal(r_sb[:], rowsum_sb[:])
                nc.vector.tensor_copy(r_bf[:], r_sb[:])
                nc.vector.tensor_scalar_mul(r128_sb[:], r_sb[:], SM)

                # ---- M8 = fp8(SM * r_j * E) (vector, overlaps Y1 gather) ----
                for jt in range(JT):
                    nc.vector.tensor_scalar_mul(
                        M8_sb[:, jt, :],
                        M_sb[:, jt * NL:(jt + 1) * NL],
                        r128_sb[:, jt:jt + 1],
                    )

                # ---- xmT = relu(wmap^T x^T + b) ----
                for et in range(ET):
                    mp = psB.tile([P, NL], F32, tag="mp", name=f"mp{et}")
                    for kt in range(KC):
                        nc.tensor.matmul(
                            mp[:],
                            wm_sb[:, kt * H + et * P: kt * H + (et + 1) * P],
                            xT_sb[:, kt * NL:(kt + 1) * NL],
                            start=(kt == 0), stop=(kt == KC - 1),
                        )
                    nc.scalar.activation(
                        xmT_sb[:, et * NL:(et + 1) * NL], mp[:], AF.Relu,
                        bias=bmap_sb[:, et:et + 1],
                    )

                # ---- XW1 = xm @ w1 (bf16 local copy; fp8 cast once dinv up)
                for q in range(2):
                    for it in range(IT):
                        wp = psB.tile([P, HQ], F32, tag="mp", name=f"wp{q}{it}")
                        for kt in range(KC):
                            nc.tensor.matmul(
                                wp[:],
                                xmT_sb[:, kt * NL + it * P: kt * NL + (it + 1) * P],
                                w1_sb[:, kt * H + q * HQ: kt * H + (q + 1) * HQ],
                                start=(kt == 0), stop=(kt == KC - 1),
                            )
                        nc.vector.tensor_copy(
                            xw1bf_sb[:, it * H + q * HQ: it * H + (q + 1) * HQ],
                            wp[:],
                        )

                # ---- degs/dinv: colsum(r_j E) + 1, local only ----
                csp = psC.tile([1, NL], F32)
                for jt in range(JT):
                    nc.tensor.matmul(
                        csp[:],
                        r_bf[:, jt:jt + 1],
                        M_sb[:, jt * NL:(jt + 1) * NL],
                        start=(jt == 0), stop=(jt == JT - 1),
                    )
                # sqrt on the 1-partition row, then broadcast/transpose and
                # take reciprocals on fully-parallel 128-partition tiles
                # (a [1, 512] DVE reciprocal costs ~3.3us; [128, *] is fast)
                nc.scalar.activation(sq_loc[:], csp[:], AF.Sqrt, bias=1.0)
                nc.sync.dma_start(dv_dram[:], sq_loc[:])
                nc.sync.dma_start(
                    sqT[:], dv_dram.rearrange("(t p) -> p t", p=P)
                )
                nc.gpsimd.dma_start(
                    sq_rep[:], dv_dram[None, :].to_broadcast((P, NL))
                )
                nc.vector.reciprocal(dinvT[:], sqT[:])
                nc.vector.reciprocal(dinv_rep[:], sq_rep[:])

                # ---- Y1 = fp8(SY1 * dinv_i * XW1), write out, AllGather ----
                for it in range(IT):
                    nc.vector.tensor_scalar(
                        y1q8_sb[:, it, :],
                        xw1bf_sb[:, it * H:(it + 1) * H],
                        dinvT[:, it:it + 1], SY1,
                        op0=OP.mult, op1=OP.mult,
                    )
                for q in range(2):
                    nc.scalar.dma_start(
                        y1_in[q].rearrange("(t p) w -> p t w", p=P),
                        y1q8_sb[:, :, q * HQ:(q + 1) * HQ],
                    )
                for q in range(2):
                    collective(y1_in[q][:], y1_out[q][:])

            # ---------- conv passes (fp8 DoubleRow) ----------
            def conv_pass(mts, slab_pool, ps_pool, y_out_q, width,
                          off_f, yq8, tagp):
                """psum[mt] = M8^T @ Ygathered + SM * Y_self, DoubleRow fp8."""
                psums = {
                    mt: ps_pool.tile([P, NL], F32, tag=f"{tagp}{mt}",
                                     name=f"{tagp}{mt}")
                    for mt in mts
                }
                SK = 8                      # k-tiles per slab super-load
                for s in range(JT // SK):
                    slab = slab_pool.tile([P, SK, width], F8, tag=f"sl{tagp}",
                                          name=f"sl{tagp}{s}")
                    nc.sync.dma_start(
                        slab[:],
                        y_out_q[s * SK * P:(s + 1) * SK * P, :]
                        .rearrange("(t p) w -> p t w", p=P),
                    )
                    for kp in range(SK // 2):
                        for mt in mts:
                            fo = mt * P - off_f
                            nc.tensor.matmul(
                                psums[mt][:],
                                slab[:, 2 * kp:2 * kp + 2, fo:fo + P],
                                M8_sb[:, s * SK + 2 * kp:s * SK + 2 * kp + 2, :],
                                start=(s == 0 and kp == 0), stop=False,
                                perf_mode=DR,
                            )
                # mt-major so psums stop one by one and epilogues overlap
                for mt in mts:
                    for tp in range(IT // 2):
                        nc.tensor.matmul(
                            psums[mt][:],
                            yq8[:, 2 * tp:2 * tp + 2, mt * P:(mt + 1) * P],
                            diag8[:, 2 * tp:2 * tp + 2, :],
                            start=False, stop=(tp == IT // 2 - 1),
                            perf_mode=DR,
                        )
                return psums

            def conv_epilogue(mts, psums, etmp_pool, b_sb, s_sb, t_sb, hT,
                              inv_scale, tagp, pool_out=None):
                for mt in mts:
                    ta = etmp_pool.tile([P, NL], F32, tag=f"ea{tagp}",
                                        name=f"ea{tagp}{mt}")
                    nc.vector.tensor_mul(ta[:], psums[mt][:], dinv_rep[:])
                    tb = etmp_pool.tile([P, NL], F32, tag=f"eb{tagp}",
                                        name=f"eb{tagp}{mt}")
                    nc.scalar.activation(
                        tb[:], ta[:], AF.Relu,
                        bias=b_sb[:, mt:mt + 1], scale=inv_scale,
                    )
                    nc.vector.tensor_scalar(
                        hT[:, mt * NL:(mt + 1) * NL], tb[:],
                        s_sb[:, mt:mt + 1], t_sb[:, mt:mt + 1],
                        op0=OP.mult, op1=OP.add,
                        accum_out=(
                            None if pool_out is None
                            else pool_out[:, mt:mt + 1]
                        ),
                    )

            with ExitStack() as cctx:
                slab_pool = cctx.enter_context(tc.tile_pool(name="slab1", bufs=4))
                etmp = cctx.enter_context(tc.tile_pool(name="etmp", bufs=2))
                for q in range(2):
                    with ExitStack() as cq:
                        psq = cq.enter_context(
                            tc.tile_pool(name=f"ps1{q}", bufs=1, space="PSUM")
                        )
                        mts = range(4 * q, 4 * q + 4)
                        psums = conv_pass(mts, slab_pool, psq, y1_out[q],
                                          HQ, q * HQ, y1q8_sb, f"c1{q}")
                        conv_epilogue(mts, psums, etmp, b1_sb, s1_sb,
                                      t1_sb, h1T_sb, 1.0 / (SM * SY1), f"1{q}")

                # ---- Y2 = fp8(SY2 * dinv_i * (h1 @ w2)), single AllGather --
                with ExitStack() as dctx:
                    ps2 = dctx.enter_context(
                        tc.tile_pool(name="ps2", bufs=2, space="PSUM")
                    )
                    for it in range(IT):
                        wp2 = ps2.tile([P, CO], F32, tag="wp2", name=f"wp2{it}")
                        for kt in range(ET):
                            nc.tensor.matmul(
                                wp2[:],
                                h1T_sb[:, kt * NL + it * P: kt * NL + (it + 1) * P],
                                w2_sb[:, kt * CO:(kt + 1) * CO],
                                start=(kt == 0), stop=(kt == ET - 1),
                            )
                        nc.vector.tensor_scalar(
                            y2q8_sb[:, it, :], wp2[:],
                            dinvT[:, it:it + 1], SY2,
                            op0=OP.mult, op1=OP.mult,
                        )
                        nc.scalar.dma_start(
                            y2_in[it * P:(it + 1) * P, :],
                            y2q8_sb[:, it, :],
                        )
                    collective(y2_in[:], y2_out[:])

            # ---------- conv2 (single pass) + mean-pool ----------
            with ExitStack() as ectx:
                slab2_pool = ectx.enter_context(tc.tile_pool(name="slab2", bufs=4))
                etmp2 = ectx.enter_context(tc.tile_pool(name="etmp2", bufs=2))
                ps3 = ectx.enter_context(
                    tc.tile_pool(name="ps3", bufs=1, space="PSUM")
                )
                mts = range(GT)
                psums = conv_pass(mts, slab2_pool, ps3, y2_out,
                                  CO, 0, y2q8_sb, "c2")
                conv_epilogue(mts, psums, etmp2, b2_sb, s2_sb, t2_sb,
                              h1T_sb, 1.0 / (SM * SY2), "2",
                              pool_out=pool_part)

            # per-core pooled partial out; host reduces across cores
            nc.gpsimd.dma_start(out_d[:], pool_part[:])

        # pin the CC stream order: rs, y1_h0, y1_h1, y2
        for a, b in zip(cc_insts[1:], cc_insts[:-1]):
            add_dep_helper(a.ins, b.ins, True, "cc stream order")

    nc.compile()
    return nc


_NC_CACHE = {}


def _get_nc():
    if "nc" not in _NC_CACHE:
        _NC_CACHE["nc"] = build()
    return _NC_CACHE["nc"]


def make_in_maps(inputs):
    import ml_dtypes

    f = np.float32
    bf = ml_dtypes.bfloat16
    x = np.asarray(inputs["x"], dtype=f)
    w_map = np.asarray(inputs["w_map"], dtype=f)
    w1 = np.asarray(inputs["w1"], dtype=f)
    w2 = np.asarray(inputs["w2"], dtype=f)
    nv1 = np.asarray(inputs["nv1"], dtype=f)
    nv2 = np.asarray(inputs["nv2"], dtype=f)

    def vec_t(v, nt):
        return np.ascontiguousarray(np.asarray(v, dtype=f).reshape(nt, P).T)

    s1 = (np.asarray(inputs["bn1_g"], f)
          / np.sqrt(np.asarray(inputs["bn1_v"], f) + BN_EPS))
    t1 = np.asarray(inputs["bn1_b"], f) - np.asarray(inputs["bn1_m"], f) * s1
    s2 = (np.asarray(inputs["bn2_g"], f)
          / np.sqrt(np.asarray(inputs["bn2_v"], f) + BN_EPS))
    t2 = np.asarray(inputs["bn2_b"], f) - np.asarray(inputs["bn2_m"], f) * s2

    common = {
        "wmap": np.ascontiguousarray(w_map.astype(bf)),
        "w1": np.ascontiguousarray(w1.astype(bf)),
        "w2": np.ascontiguousarray(w2.astype(bf)),
        "nv1T": np.ascontiguousarray(nv1.T.astype(bf)),
        "bmap_t": vec_t(inputs["b_map"], ET),
        "b1_t": vec_t(inputs["b1"], ET),
        "s1_t": vec_t(s1, ET),
        "t1_t": vec_t(t1, ET),
        "b2_t": vec_t(inputs["b2"], GT),
        "s2_t": vec_t(s2, GT),
        "t2_t": vec_t(t2, GT),
    }
    in_maps = []
    for c in range(NCORES):
        m = dict(common)
        m["xT"] = np.ascontiguousarray(x[c * NL:(c + 1) * NL].T.astype(bf))
        m["nv2s"] = np.ascontiguousarray(nv2[:, c * NL:(c + 1) * NL].astype(bf))
        in_maps.append(m)
    return in_maps


def finish_host(results, inputs):
    """Sum per-core pooled partials, apply mean + attention gate."""
    f = np.float32
    pooled_sum = np.zeros(CO, f)
    for res in results:
        arr = np.asarray(res["out"], dtype=f)      # [P, GT], g = t*P + p
        pooled_sum += arr.T.reshape(-1)
    pooled = pooled_sum / N
    w_attn = np.asarray(inputs["w_attn"], f).reshape(-1)
    b_attn = np.asarray(inputs["b_attn"], f).reshape(-1)[0]
    z = float(pooled @ w_attn + b_attn)
    attn = 1.0 / (1.0 + np.exp(-z))
    return (pooled * attn)[None, :].astype(f)


def run(inputs, trace=False, tmpdir=None):
    nc = _get_nc()
    in_maps = make_in_maps(inputs)
    res = run_bass_kernel_spmd(
        nc, in_maps, core_ids=list(range(NCORES)), trace=trace, tmpdir=tmpdir
    )
    out = finish_host(res.results, inputs)
    return out, res


def kernel(**inputs):
    out, _ = run(inputs)
    return out


# revision 19
# speedup vs baseline: 1.3677x; 1.0150x over previous
"""Distributed Trainium2 Bass kernel for AdaptiveGCN (N=4096, CIN=1024, H=1024, COUT=512, R=10).

Sharding: node dimension split across 8 NeuronCores (512 nodes/core).
Each core owns a 512-column block of the dense adaptive adjacency and the
matching 512 output rows.

v2 layout of the math (per core):
  E        = exp(relu(nv1 @ nv2))            column block, bf16
  r_j      = 1 / rowsum(E)                   (rowsum partials AllGathered)
  M8       = fp8(128 * r_j * E)              adjacency block for fp8 DoubleRow
  deg_i    = colsum(r_j E) + 1  (local),  dinv_i = deg_i^-1/2   (local, NO gather)
  Y1       = fp8(4 * dinv_i * (xm @ w1))     scaled BEFORE the AllGather, fp8 wire
  conv1    = relu((M8^T @ Y1 + 128*Y1_self) * dinv_i / 512 + b1) -> bn1 -> h1
  Y2       = fp8(8 * dinv_i * (h1 @ w2))     single fp8 AllGather
  conv2    = same shape, then mean-pool partials returned per core.

Collectives (CC stream order): rowsum partials (16KB f32), Y1 in two fp8
halves (256KB each), Y2 in one fp8 shot (256KB). The dinv gather of the
baseline is gone (dinv is folded into Y before the gather).

kernel(**inputs) takes the FULL unsharded inputs (same keys as
reference.setup_inputs()) and returns the FULL [1, 512] float32 output.
"""

import os
import sys
from contextlib import ExitStack

import numpy as np

for _p in ("/opt/trn_rl_repo", "/root/.axon_site/_ro/trn_rl_repo"):
    if os.path.isdir(_p) and _p not in sys.path:
        sys.path.insert(0, _p)

import concourse.bass as bass
import concourse.bacc as bacc
import concourse.tile as tile
from concourse import mybir
from concourse.bass_utils import run_bass_kernel_spmd
from concourse.masks import make_identity
from concourse.tile_rust import add_dep_helper

F32 = mybir.dt.float32
BF16 = mybir.dt.bfloat16
F8 = mybir.dt.float8e4
AF = mybir.ActivationFunctionType
OP = mybir.AluOpType
AX = mybir.AxisListType
DR = mybir.MatmulPerfMode.DoubleRow

NCORES = 8
N = 4096
NL = N // NCORES          # 512 nodes per core
CIN = 1024
H = 1024
CO = 512
R = 10
P = 128
JT = N // P               # 32 j-tiles
ET = H // P               # 8
IT = NL // P              # 4 local-node tiles
GT = CO // P              # 4
KC = CIN // P             # 8 cin k-tiles
BN_EPS = 1e-5
HQ = H // 2               # 512: Y1 AG half width
SM = 128.0                # fp8 scale on the adjacency block
SY1 = 4.0                 # fp8 scale on Y1
SY2 = 8.0                 # fp8 scale on Y2


def build():
    """Build the SPMD Bass graph (identical on all 8 cores)."""
    nc = bacc.Bacc(None, target_bir_lowering=False, debug=False, num_devices=NCORES)

    # ---- external parameters (per-core shards / replicated) ----
    xT_d = nc.declare_dram_parameter("xT", [CIN, NL], BF16, isOutput=False)
    wmap_d = nc.declare_dram_parameter("wmap", [CIN, H], BF16, isOutput=False)
    w1_d = nc.declare_dram_parameter("w1", [H, H], BF16, isOutput=False)
    w2_d = nc.declare_dram_parameter("w2", [H, CO], BF16, isOutput=False)
    nv1T_d = nc.declare_dram_parameter("nv1T", [R, N], BF16, isOutput=False)
    nv2s_d = nc.declare_dram_parameter("nv2s", [R, NL], BF16, isOutput=False)
    bmap_d = nc.declare_dram_parameter("bmap_t", [P, ET], F32, isOutput=False)
    b1_d = nc.declare_dram_parameter("b1_t", [P, ET], F32, isOutput=False)
    s1_d = nc.declare_dram_parameter("s1_t", [P, ET], F32, isOutput=False)
    t1_d = nc.declare_dram_parameter("t1_t", [P, ET], F32, isOutput=False)
    b2_d = nc.declare_dram_parameter("b2_t", [P, GT], F32, isOutput=False)
    s2_d = nc.declare_dram_parameter("s2_t", [P, GT], F32, isOutput=False)
    t2_d = nc.declare_dram_parameter("t2_t", [P, GT], F32, isOutput=False)
    out_d = nc.declare_dram_parameter("out", [P, GT], F32, isOutput=True)

    # ---- internal DRAM: collective bounce buffers + dinv scratch ----
    rg = [list(range(NCORES))]
    rs_in = nc.dram_tensor("rs_in", [P, JT], F32)
    rs_out = nc.dram_tensor("rs_out", [P, JT], F32, addr_space="Shared")
    y1_in = [nc.dram_tensor(f"y1_in{q}", [NL, HQ], F8) for q in range(2)]
    y1_out = [
        nc.dram_tensor(f"y1_out{q}", [N, HQ], F8, addr_space="Shared")
        for q in range(2)
    ]
    y2_in = nc.dram_tensor("y2_in", [NL, CO], F8)
    y2_out = nc.dram_tensor("y2_out", [N, CO], F8, addr_space="Shared")
    dv_dram = nc.dram_tensor("dv_dram", [NL], F32)

    cc_insts = []

    def collective(in_ap, out_ap, kind="AllGather", op=OP.bypass):
        cc = nc.gpsimd.collective_compute(
            kind, op, replica_groups=rg,
            ins=[in_ap], outs=[out_ap],
        )
        cc_insts.append(cc)
        return cc

    with tile.TileContext(nc) as tc:
        with ExitStack() as ctx:
            # ---------- persistent pool ----------
            pp = ctx.enter_context(tc.tile_pool(name="persist", bufs=1))

            nv1T_sb = pp.tile([R, N], BF16)
            nv2s_sb = pp.tile([R, NL], BF16)
            nc.sync.dma_start(nv1T_sb[:], nv1T_d[:])
            nc.sync.dma_start(nv2s_sb[:], nv2s_d[:])

            xT_sb = pp.tile([P, KC * NL], BF16)
            wm_sb = pp.tile([P, KC * H], BF16)
            w1_sb = pp.tile([P, KC * H], BF16)
            nc.sync.dma_start(
                xT_sb[:].rearrange("p (t w) -> p t w", w=NL),
                xT_d.rearrange("(t p) w -> p t w", p=P),
            )
            nc.sync.dma_start(
                wm_sb[:].rearrange("p (t w) -> p t w", w=H),
                wmap_d.rearrange("(t p) w -> p t w", p=P),
            )
            nc.sync.dma_start(
                w1_sb[:].rearrange("p (t w) -> p t w", w=H),
                w1_d.rearrange("(t p) w -> p t w", p=P),
            )

            rs_part = pp.tile([P, JT], F32)
            rowsum_sb = pp.tile([P, JT], F32)
            r_sb = pp.tile([P, JT], F32)
            r_bf = pp.tile([P, JT], BF16)
            r128_sb = pp.tile([P, JT], F32)
            dinv_loc = pp.tile([1, NL], F32)
            dinvT = pp.tile([P, IT], F32)
            dinv_rep = pp.tile([P, NL], F32)

            bmap_sb = pp.tile([P, ET], F32)
            b1_sb = pp.tile([P, ET], F32)
            s1_sb = pp.tile([P, ET], F32)
            t1_sb = pp.tile([P, ET], F32)
            b2_sb = pp.tile([P, GT], F32)
            s2_sb = pp.tile([P, GT], F32)
            t2_sb = pp.tile([P, GT], F32)
            for sb, d in (
                (bmap_sb, bmap_d), (b1_sb, b1_d), (s1_sb, s1_d), (t1_sb, t1_d),
                (b2_sb, b2_d), (s2_sb, s2_d), (t2_sb, t2_d),
            ):
                nc.sync.dma_start(sb[:], d[:])

            M_sb = pp.tile([P, JT * NL], BF16)       # E = exp(relu(z)) block
            M8_sb = pp.tile([P, JT, NL], F8)         # fp8(SM * r_j * E)
            ident_sb = pp.tile([P, P], F32)
            ident8 = pp.tile([P, P], F8)             # SM * I
            diag8 = pp.tile([P, IT, NL], F8)         # self-loop rhs tiles
            make_identity(nc, ident_sb[:])
            nc.vector.tensor_scalar_mul(ident8[:], ident_sb[:], SM)
            nc.gpsimd.memset(diag8[:], 0.0)
            for it in range(IT):
                nc.vector.tensor_copy(
                    diag8[:, it, it * P:(it + 1) * P], ident8[:]
                )

            xmT_sb = pp.tile([P, ET * NL], BF16)     # relu(x wmap)^T, [f, i]
            xw1bf_sb = pp.tile([P, IT * H], BF16)    # local XW1, [i, f]
            y1q8_sb = pp.tile([P, IT, H], F8)        # fp8(SY1 dinv XW1)
            h1T_sb = pp.tile([P, ET * NL], BF16)     # conv1 out, [f, i]
            y2q8_sb = pp.tile([P, IT, CO], F8)       # fp8(SY2 dinv XW2)
            pool_part = pp.tile([P, GT], F32)
            w2_sb = pp.tile([P, KC * CO], BF16)
            nc.sync.dma_start(
                w2_sb[:].rearrange("p (t w) -> p t w", w=CO),
                w2_d.rearrange("(t p) w -> p t w", p=P),
            )

            with ExitStack() as actx:
                tmp_pool = actx.enter_context(tc.tile_pool(name="tmpA", bufs=4))
                psA = actx.enter_context(
                    tc.tile_pool(name="psA", bufs=3, space="PSUM")
                )
                psC = actx.enter_context(
                    tc.tile_pool(name="psC", bufs=1, space="PSUM")
                )

                psB = actx.enter_context(
                    tc.tile_pool(name="psB", bufs=2, space="PSUM")
                )

                # ---- z = nv1 @ nv2 column block; E = exp(relu(z)) + rowsum.
                # exp(relu(z)) == max(exp(z), 1): scalar exp straight off
                # PSUM, vector does the max at 2x bf16 rate with the row-sum
                # accumulated in the same pass.
                for jt in range(JT):
                    zp = psA.tile([P, NL], F32, tag="zp", name=f"zp{jt}")
                    nc.tensor.matmul(
                        zp[:],
                        nv1T_sb[:, jt * P:(jt + 1) * P],
                        nv2s_sb[:],
                        start=True, stop=True,
                    )
                    ez = tmp_pool.tile([P, NL], BF16, tag="ez", name=f"ez{jt}")
                    nc.scalar.activation(ez[:], zp[:], AF.Exp)
                    # with accum_out, op1 is the REDUCE op: dst = max(in, 1),
                    # accum = rowsum(dst) + scalar2
                    nc.vector.tensor_scalar(
                        M_sb[:, jt * NL:(jt + 1) * NL], ez[:], 1.0, 0.0,
                        op0=OP.max, op1=OP.add,
                        accum_out=rs_part[:, jt:jt + 1],
                    )

                # ---- AllReduce softmax row-sum partials (fabric adds) ----
                nc.gpsimd.dma_start(rs_in[:], rs_part[:])
                collective(rs_in[:], rs_out[:], kind="AllReduce", op=OP.add)
                nc.gpsimd.dma_start(rowsum_sb[:], rs_out[:])
                nc.vector.reciprocal(r_sb[:], rowsum_sb[:])
                nc.vector.tensor_copy(r_bf[:], r_sb[:])
                nc.vector.tensor_scalar_mul(r128_sb[:], r_sb[:], SM)

                # ---- M8 = fp8(SM * r_j * E) (vector, overlaps Y1 gather) ----
                for jt in range(JT):
                    nc.vector.tensor_scalar_mul(
                        M8_sb[:, jt, :],
                        M_sb[:, jt * NL:(jt + 1) * NL],
                        r128_sb[:, jt:jt + 1],
                    )

                # ---- xmT = relu(wmap^T x^T + b) ----
                for et in range(ET):
                    mp = psB.tile([P, NL], F32, tag="mp", name=f"mp{et}")
                    for kt in range(KC):
                        nc.tensor.matmul(
                            mp[:],
                            wm_sb[:, kt * H + et * P: kt * H + (et + 1) * P],
                            xT_sb[:, kt * NL:(kt + 1) * NL],
                            start=(kt == 0), stop=(kt == KC - 1),
                        )
                    nc.scalar.activation(
                        xmT_sb[:, et * NL:(et + 1) * NL], mp[:], AF.Relu,
                        bias=bmap_sb[:, et:et + 1],
                    )

                # ---- XW1 = xm @ w1 (bf16 local copy; fp8 cast once dinv up)
                for q in range(2):
                    for it in range(IT):
                        wp = psB.tile([P, HQ], F32, tag="mp", name=f"wp{q}{it}")
                        for kt in range(KC):
                            nc.tensor.matmul(
                                wp[:],
                                xmT_sb[:, kt * NL + it * P: kt * NL + (it + 1) * P],
                                w1_sb[:, kt * H + q * HQ: kt * H + (q + 1) * HQ],
                                start=(kt == 0), stop=(kt == KC - 1),
                            )
                        nc.vector.tensor_copy(
                            xw1bf_sb[:, it * H + q * HQ: it * H + (q + 1) * HQ],
                            wp[:],
                        )

                # ---- degs/dinv: colsum(r_j E) + 1, local only ----
                csp = psC.tile([1, NL], F32)
                for jt in range(JT):
                    nc.tensor.matmul(
                        csp[:],
                        r_bf[:, jt:jt + 1],
                        M_sb[:, jt * NL:(jt + 1) * NL],
                        start=(jt == 0), stop=(jt == JT - 1),
                    )
                # dinv = (deg+1)^-1/2 in one scalar LUT op off PSUM, then
                # broadcast/transpose the finished values via DMA
                nc.scalar.activation(
                    dinv_loc[:], csp[:], AF.Abs_reciprocal_sqrt, bias=1.0
                )
                nc.sync.dma_start(dv_dram[:], dinv_loc[:])
                nc.sync.dma_start(
                    dinvT[:], dv_dram.rearrange("(t p) -> p t", p=P)
                )
                nc.gpsimd.dma_start(
                    dinv_rep[:], dv_dram[None, :].to_broadcast((P, NL))
                )

                # ---- Y1 = fp8(SY1 * dinv_i * XW1), write out, AllGather ----
                for it in range(IT):
                    eng = nc.vector if it % 2 == 0 else nc.gpsimd
                    eng.tensor_scalar(
                        y1q8_sb[:, it, :],
                        xw1bf_sb[:, it * H:(it + 1) * H],
                        dinvT[:, it:it + 1], SY1,
                        op0=OP.mult, op1=OP.mult,
                    )
                for q in range(2):
                    nc.scalar.dma_start(
                        y1_in[q].rearrange("(t p) w -> p t w", p=P),
                        y1q8_sb[:, :, q * HQ:(q + 1) * HQ],
                    )
                for q in range(2):
                    collective(y1_in[q][:], y1_out[q][:])

            # ---------- conv passes (fp8 DoubleRow) ----------
            def conv_pass(mts, slab_pool, ps_pool, y_out_q, width,
                          off_f, yq8, tagp):
                """psum[mt] = M8^T @ Ygathered + SM * Y_self, DoubleRow fp8."""
                psums = {
                    mt: ps_pool.tile([P, NL], F32, tag=f"{tagp}{mt}",
                                     name=f"{tagp}{mt}")
                    for mt in mts
                }
                # self-loop first: doesn't need the gathered slabs, so it
                # runs while the AllGather is still in flight
                for mt in mts:
                    for tp in range(IT // 2):
                        nc.tensor.matmul(
                            psums[mt][:],
                            yq8[:, 2 * tp:2 * tp + 2, mt * P:(mt + 1) * P],
                            diag8[:, 2 * tp:2 * tp + 2, :],
                            start=(tp == 0), stop=False,
                            perf_mode=DR,
                        )
                # small first slab so matmuls start right after the gather
                splits = [2, 8, 8, 8, 6]
                kt0 = 0
                for si, sk in enumerate(splits):
                    slab = slab_pool.tile([P, sk, width], F8, tag=f"sl{tagp}{si}",
                                          name=f"sl{tagp}{si}", bufs=1)
                    nc.sync.dma_start(
                        slab[:],
                        y_out_q[kt0 * P:(kt0 + sk) * P, :]
                        .rearrange("(t p) w -> p t w", p=P),
                    )
                    last = (si == len(splits) - 1)
                    for kp in range(sk // 2):
                        for mt in mts:
                            fo = mt * P - off_f
                            nc.tensor.matmul(
                                psums[mt][:],
                                slab[:, 2 * kp:2 * kp + 2, fo:fo + P],
                                M8_sb[:, kt0 + 2 * kp:kt0 + 2 * kp + 2, :],
                                start=False,
                                stop=(last and kp == sk // 2 - 1),
                                perf_mode=DR,
                            )
                    kt0 += sk
                return psums

            def conv_epilogue(mts, psums, etmp_pool, b_sb, s_sb, t_sb, hT,
                              inv_scale, tagp, pool_out=None):
                for mt in mts:
                    ta = etmp_pool.tile([P, NL], F32, tag=f"ea{tagp}",
                                        name=f"ea{tagp}{mt}")
                    nc.vector.tensor_mul(ta[:], psums[mt][:], dinv_rep[:])
                    tb = etmp_pool.tile([P, NL], F32, tag=f"eb{tagp}",
                                        name=f"eb{tagp}{mt}")
                    nc.scalar.activation(
                        tb[:], ta[:], AF.Relu,
                        bias=b_sb[:, mt:mt + 1], scale=inv_scale,
                    )
                    nc.vector.tensor_scalar(
                        hT[:, mt * NL:(mt + 1) * NL], tb[:],
                        s_sb[:, mt:mt + 1], t_sb[:, mt:mt + 1],
                        op0=OP.mult, op1=OP.add,
                        accum_out=(
                            None if pool_out is None
                            else pool_out[:, mt:mt + 1]
                        ),
                    )

            with ExitStack() as cctx:
                slab_pool = cctx.enter_context(tc.tile_pool(name="slab1", bufs=4))
                etmp = cctx.enter_context(tc.tile_pool(name="etmp", bufs=2))
                for q in range(2):
                    with ExitStack() as cq:
                        psq = cq.enter_context(
                            tc.tile_pool(name=f"ps1{q}", bufs=1, space="PSUM")
                        )
                        mts = range(4 * q, 4 * q + 4)
                        psums = conv_pass(mts, slab_pool, psq, y1_out[q],
                                          HQ, q * HQ, y1q8_sb, f"c1{q}")
                        conv_epilogue(mts, psums, etmp, b1_sb, s1_sb,
                                      t1_sb, h1T_sb, 1.0 / (SM * SY1), f"1{q}")

                # ---- Y2 = fp8(SY2 * dinv_i * (h1 @ w2)), single AllGather --
                with ExitStack() as dctx:
                    ps2 = dctx.enter_context(
                        tc.tile_pool(name="ps2", bufs=2, space="PSUM")
                    )
                    for it in range(IT):
                        wp2 = ps2.tile([P, CO], F32, tag="wp2", name=f"wp2{it}")
                        for kt in range(ET):
                            nc.tensor.matmul(
                                wp2[:],
                                h1T_sb[:, kt * NL + it * P: kt * NL + (it + 1) * P],
                                w2_sb[:, kt * CO:(kt + 1) * CO],
                                start=(kt == 0), stop=(kt == ET - 1),
                            )
                        nc.vector.tensor_scalar(
                            y2q8_sb[:, it, :], wp2[:],
                            dinvT[:, it:it + 1], SY2,
                            op0=OP.mult, op1=OP.mult,
                        )
                        nc.scalar.dma_start(
                            y2_in[it * P:(it + 1) * P, :],
                            y2q8_sb[:, it, :],
                        )
                    collective(y2_in[:], y2_out[:])

            # ---------- conv2 (single pass) + mean-pool ----------
            with ExitStack() as ectx:
                slab2_pool = ectx.enter_context(tc.tile_pool(name="slab2", bufs=4))
                etmp2 = ectx.enter_context(tc.tile_pool(name="etmp2", bufs=2))
                ps3 = ectx.enter_context(
                    tc.tile_pool(name="ps3", bufs=1, space="PSUM")
                )
                mts = range(GT)
                psums = conv_pass(mts, slab2_pool, ps3, y2_out,
                                  CO, 0, y2q8_sb, "c2")
                conv_epilogue(mts, psums, etmp2, b2_sb, s2_sb, t2_sb,
                              h1T_sb, 1.0 / (SM * SY2), "2",
                              pool_out=pool_part)

            # per-core pooled partial out; host reduces across cores
            nc.gpsimd.dma_start(out_d[:], pool_part[:])

        # pin the CC stream order: rs, y1_h0, y1_h1, y2
        for a, b in zip(cc_insts[1:], cc_insts[:-1]):
            add_dep_helper(a.ins, b.ins, True, "cc stream order")

    nc.compile()
    return nc


_NC_CACHE = {}


def _get_nc():
    if "nc" not in _NC_CACHE:
        _NC_CACHE["nc"] = build()
    return _NC_CACHE["nc"]


def make_in_maps(inputs):
    import ml_dtypes

    f = np.float32
    bf = ml_dtypes.bfloat16
    x = np.asarray(inputs["x"], dtype=f)
    w_map = np.asarray(inputs["w_map"], dtype=f)
    w1 = np.asarray(inputs["w1"], dtype=f)
    w2 = np.asarray(inputs["w2"], dtype=f)
    nv1 = np.asarray(inputs["nv1"], dtype=f)
    nv2 = np.asarray(inputs["nv2"], dtype=f)

    def vec_t(v, nt):
        return np.ascontiguousarray(np.asarray(v, dtype=f).reshape(nt, P).T)

    s1 = (np.asarray(inputs["bn1_g"], f)
          / np.sqrt(np.asarray(inputs["bn1_v"], f) + BN_EPS))
    t1 = np.asarray(inputs["bn1_b"], f) - np.asarray(inputs["bn1_m"], f) * s1
    s2 = (np.asarray(inputs["bn2_g"], f)
          / np.sqrt(np.asarray(inputs["bn2_v"], f) + BN_EPS))
    t2 = np.asarray(inputs["bn2_b"], f) - np.asarray(inputs["bn2_m"], f) * s2

    common = {
        "wmap": np.ascontiguousarray(w_map.astype(bf)),
        "w1": np.ascontiguousarray(w1.astype(bf)),
        "w2": np.ascontiguousarray(w2.astype(bf)),
        "nv1T": np.ascontiguousarray(nv1.T.astype(bf)),
        "bmap_t": vec_t(inputs["b_map"], ET),
        "b1_t": vec_t(inputs["b1"], ET),
        "s1_t": vec_t(s1, ET),
        "t1_t": vec_t(t1, ET),
        "b2_t": vec_t(inputs["b2"], GT),
        "s2_t": vec_t(s2, GT),
        "t2_t": vec_t(t2, GT),
    }
    in_maps = []
    for c in range(NCORES):
        m = dict(common)
        m["xT"] = np.ascontiguousarray(x[c * NL:(c + 1) * NL].T.astype(bf))
        m["nv2s"] = np.ascontiguousarray(nv2[:, c * NL:(c + 1) * NL].astype(bf))
        in_maps.append(m)
    return in_maps


def finish_host(results, inputs):
    """Sum per-core pooled partials, apply mean + attention gate."""
    f = np.float32
    pooled_sum = np.zeros(CO, f)
    for res in results:
        arr = np.asarray(res["out"], dtype=f)      # [P, GT], g = t*P + p
        pooled_sum += arr.T.reshape(-1)
    pooled = pooled_sum / N
    w_attn = np.asarray(inputs["w_attn"], f).reshape(-1)
    b_attn = np.asarray(inputs["b_attn"], f).reshape(-1)[0]
    z = float(pooled @ w_attn + b_attn)
    attn = 1.0 / (1.0 + np.exp(-z))
    return (pooled * attn)[None, :].astype(f)


def run(inputs, trace=False, tmpdir=None):
    nc = _get_nc()
    in_maps = make_in_maps(inputs)
    res = run_bass_kernel_spmd(
        nc, in_maps, core_ids=list(range(NCORES)), trace=trace, tmpdir=tmpdir
    )
    out = finish_host(res.results, inputs)
    return out, res


def kernel(**inputs):
    out, _ = run(inputs)
    return out
